# revision 9
# baseline (speedup 1.0000x reference)
"""Trainium2 Bass kernel for nn_AutoEncoder (segment_reduce).

6-layer MLP autoencoder on a single 16384-vector + segmented softmax over
1024 contiguous segments, distributed over 8 NeuronCores.

v4 design (vs the v2 CC-AllReduce baseline at ~161us):
  * All mat-vec layers run WEIGHT-STATIONARY: lhsT = 128x128 fp8/bf16
    weight block (Fast Weight Load eligible), rhs = the activation column
    [128, 1]. A weight-load-dominated fp8 matmul is ~40ns vs ~N cycles
    when the weights stream as the moving operand (the v2 scheme cost
    ~183ns per 256 streamed columns). L1 = 256 matmuls, L6 = 288.
  * NO y AllReduce (was 27us of CC time + 23us of DRAM-readback tail).
    Instead the tiny h5 vector (2048 f32) is all-gathered P2P: each core
    broadcasts its 256-value shard ([128, 2] f32, 1KB) to all 7 peers,
    then computes y EXACTLY on its own haloed 2304-column span of W6
    (column-sharded decoder). W6 rows per core are host-permuted to the
    XOR-delta arrival order sigma(c, d) = BASE[BASE[c]^d], so the shared
    SPMD instruction stream indexes receive slots uniformly.
  * h2 partial exchange collapsed from a 2-round XOR tree to a single
    round of 7 broadcasts (one latency, 14KB on the wire).
  * x is pre-transposed on the host ([128, 128] bf16 column layout), so
    no on-device PE transpose for it.
  * Softmax runs straight from SBUF ([128, 18] -> PE transpose -> the
    proven windowed-scan pipeline); output is each core's own 2048-slice.

Cross-box safety: logical delivery is D(j,d) = BASE[BASE[j]^d] with
BASE = (0,1,2,3,6,7,4,5). Ultra-pod die-flips XOR the whole real-NC table
by a constant, which cancels in D — verified empirically on this box.

Sem-wait handling: Tile's scheduling sim cannot model waits satisfied only
by peers, so remote-sem waits are injected post-Tile onto marker NoOps.
Receive slots are never written locally (no memset) to avoid racing
early-arriving peer data.
"""

import sys

if "/opt/trn_rl_repo" not in sys.path:
    sys.path.insert(0, "/opt/trn_rl_repo")

import numpy as np

import concourse.bass as bass
import concourse.mybir as mybir
import concourse.tile as tile
from concourse import library_config
from concourse.bass_utils import run_bass_kernel_spmd
from concourse.library_overlay import lower_extended_insts
from concourse.tile_rust import add_dep_helper

DS = 16384
H1, H2, H3 = 2048, 512, 128
NC = 8
C1 = H1 // NC       # 256  h1 / h5 shard
SLICE = DS // NC    # 2048 y-slice per core
HALO = 128
SPAN = SLICE + 2 * HALO          # 2304
NQ = SPAN // 128                 # 18 chunks of 128
KC1 = DS // 128                  # 128 k-chunks for L1
KC6 = H1 // 128                  # 16 k-chunks for L6
F32 = mybir.dt.float32
BF16 = mybir.dt.bfloat16
FP8 = mybir.dt.float8e4
FP8_SCALE = 64.0
BASE = (0, 1, 2, 3, 6, 7, 4, 5)  # logical -> real NC (global XOR offsets cancel)


def _split_sync_waits(nc):
    """The walrus build in this env only allows 1 sync wait on CTRL-class
    instructions (Drain/NoOp). Tile's tail drain carries one wait per live
    semaphore lane. Split excess waits onto preceding single-wait NOPs."""
    for f in nc.m.functions:
        for b in f.blocks:
            new_insts = []
            for inst in b.instructions:
                si = inst.sync_info
                if si is not None and si.on_wait and len(si.on_wait) > 1:
                    waits = list(si.on_wait)
                    head, tail = waits[:-1], waits[-1:]
                    for i, w in enumerate(head):
                        new_insts.append(
                            mybir.InstNoOp(
                                name=f"{inst.name}-ws{i}",
                                engine=inst.engine,
                                bass_nofuse=True,
                                sync_info=mybir.SyncInfo(on_wait=[w], on_update=[]),
                            )
                        )
                    si.on_wait = tail
                new_insts.append(inst)
            b.instructions = new_insts


def build_graph():
    nc = bass.Bass(num_swdge_queues=2)
    P = nc.declare_dram_parameter
    xT_in = P("xT", [128, 128], BF16, isOutput=False)       # xT[k, kc] = x[128kc+k]
    w1 = P("w1", [4, 128, 64 * 128], FP8, isOutput=False)   # m-major: (ct, k, kl*128+p)
    w2 = P("w2", [128, 2 * 512], BF16, isOutput=False)
    w3 = P("w3", [128, 4 * 128], BF16, isOutput=False)
    w4 = P("w4", [128, 512], BF16, isOutput=False)
    w5 = P("w5", [128, 4 * C1], BF16, isOutput=False)
    w6 = P("w6", [6, 128, 3 * KC6 * 128], FP8, isOutput=False)  # (g, k, (ql*16+kc)*128+p)
    b1c = P("b1c", [128, 2], F32, isOutput=False)
    b2c = P("b2c", [128, 4], F32, isOutput=False)
    b3c = P("b3c", [128, 1], F32, isOutput=False)
    b4c = P("b4c", [128, 4], F32, isOutput=False)
    b5c = P("b5c", [128, 2], F32, isOutput=False)
    b6q = P("b6q", [128, NQ], F32, isOutput=False)          # b6 span cols * scale
    eye = P("eye", [128, 128], F32, isOutput=False)
    jrev = P("jrev", [128, 128], F32, isOutput=False)
    jr18 = P("jr18", [NQ, NQ], F32, isOutput=False)
    shm18 = P("shm18", [NQ, NQ], F32, isOutput=False)
    mf_in = P("mf", [NQ, 256], F32, isOutput=False)
    mr_in = P("mr", [NQ, 256], F32, isOutput=False)
    out_ext = P("out", [SLICE], F32, isOutput=True)

    Tanh = mybir.ActivationFunctionType.Tanh
    Exp = mybir.ActivationFunctionType.Exp
    ADD = mybir.AluOpType.add
    SUB = mybir.AluOpType.subtract
    MUL = mybir.AluOpType.mult

    rsem2 = nc.alloc_semaphore("rsem2")
    rsem5 = nc.alloc_semaphore("rsem5")
    lsem = nc.alloc_semaphore("lsem_rdma")
    markers = {}

    with tile.TileContext(nc) as tc:
        with (
            tc.tile_pool(name="const", bufs=1) as cp,
            tc.tile_pool(name="w1p", bufs=4) as w1p,
            tc.tile_pool(name="w6p", bufs=6) as w6p,
            tc.tile_pool(name="act", bufs=1) as ap,
            tc.tile_pool(name="psA", bufs=2, space="PSUM") as psA,
            tc.tile_pool(name="ps1p", bufs=1, space="PSUM") as ps1p,
            tc.tile_pool(name="ps6p", bufs=1, space="PSUM") as ps6p,
            tc.tile_pool(name="dram", bufs=1, space="DRAM") as dp,
        ):
            # gpsimd: load the remote_dma ucode library up front (SWDGE queue
            # otherwise idle until the exchanges)
            nc.gpsimd.load_library(library_config.remote_dma)

            # Dummy CC AllGather: its presence in the NEFF forces a
            # synchronized cross-rank launch (without any CC instruction the
            # axon runtime dispatches cores milliseconds apart, which any
            # cross-core exchange then serializes on). Nothing consumes the
            # result; it also absorbs ncfw's first-collective wake latency.
            dumin = dp.tile([8], F32)
            dumout = dp.tile([8 * NC], F32, addr_space="Shared")
            nc.gpsimd.collective_compute(
                "AllGather", mybir.AluOpType.bypass,
                ins=[dumin[:].opt()], outs=[dumout[:].opt()],
                replica_groups=[list(range(NC))],
            )

            # ---- sync HWDGE ring: x first (needed earliest), then encoder
            xT = cp.tile([128, 128], BF16)
            nc.sync.dma_start(xT[:], xT_in[:])
            eyesb = cp.tile([128, 128], F32)
            nc.sync.dma_start(eyesb[:], eye[:])
            b1sb = cp.tile([128, 2], F32)
            nc.sync.dma_start(b1sb[:], b1c[:])
            w2sb = cp.tile([128, 1024], BF16)
            nc.sync.dma_start(w2sb[:], w2[:])
            w3sb = cp.tile([128, 512], BF16)
            nc.sync.dma_start(w3sb[:], w3[:])
            b2sb = cp.tile([128, 4], F32)
            nc.sync.dma_start(b2sb[:], b2c[:])
            b3sb = cp.tile([128, 1], F32)
            nc.sync.dma_start(b3sb[:], b3c[:])
            # ---- scalar ring: softmax constants + later-needed weights
            jsb = cp.tile([128, 128], F32)
            nc.scalar.dma_start(jsb[:], jrev[:])
            j18sb = cp.tile([NQ, NQ], F32)
            nc.scalar.dma_start(j18sb[:], jr18[:])
            sh18sb = cp.tile([NQ, NQ], F32)
            nc.scalar.dma_start(sh18sb[:], shm18[:])
            mf = cp.tile([NQ, 256], F32)
            nc.scalar.dma_start(mf[:], mf_in[:])
            mr = cp.tile([NQ, 256], F32)
            nc.scalar.dma_start(mr[:], mr_in[:])
            w4sb = cp.tile([128, 512], BF16)
            nc.scalar.dma_start(w4sb[:], w4[:])
            w5sb = cp.tile([128, 1024], BF16)
            nc.scalar.dma_start(w5sb[:], w5[:])
            b4sb = cp.tile([128, 4], F32)
            nc.scalar.dma_start(b4sb[:], b4c[:])
            b5sb = cp.tile([128, 2], F32)
            nc.scalar.dma_start(b5sb[:], b5c[:])
            b6sb = cp.tile([128, NQ], F32)
            nc.scalar.dma_start(b6sb[:], b6q[:])

            # ---- bulk W1 + W6 stream on SWDGE queue 0 (fast desc-gen);
            # the exchanges ride SWDGE queue 1 so they never queue behind bulk
            w1sb_l = []
            for ct in range(4):
                t = w1p.tile([128, 64 * 128], FP8, tag="w1sb", name=f"w1sb{ct}")
                nc.gpsimd.dma_start(t[:], w1[ct])
                w1sb_l.append(t)
            w6sb = []
            for g in range(6):
                t = w6p.tile([128, 3 * KC6 * 128], FP8, tag="w6", name=f"w6sb{g}")
                nc.gpsimd.dma_start(t[:], w6[g])
                w6sb.append(t)

            # ---- L1: h1 shard [128, 2] cols, weight-stationary fp8 FWL.
            # CRITICAL: each PSUM column's accumulation chain must be
            # CONTIGUOUS — interleaving two open chains on the PE corrupts
            # the accumulation (~6e-2 rel err, micro-verified). W1 is laid
            # out m-major so chains stay chunk-streamable.
            ps1 = ps1p.tile([128, 2], F32, tag="ps1", name="ps1")
            for ct in range(4):
                m = ct // 2
                for kl in range(64):
                    kc = 64 * (ct % 2) + kl
                    nc.tensor.matmul(
                        ps1[:, m : m + 1],
                        w1sb_l[ct][:, kl * 128 : (kl + 1) * 128],
                        xT[:, kc : kc + 1],
                        start=(ct % 2 == 0 and kl == 0),
                        stop=(ct % 2 == 1 and kl == 63),
                    )
            h1cols = ap.tile([128, 2], BF16)
            for m in range(2):
                nc.scalar.activation(
                    h1cols[:, m : m + 1], ps1[:, m : m + 1], Tanh,
                    bias=b1sb[:, m : m + 1], scale=1.0 / FP8_SCALE,
                )

            # ---- L2 partials ----
            ps2 = psA.tile([128, 4], F32, tag="psA", name="ps2")
            for m in range(4):
                for k in range(2):
                    nc.tensor.matmul(
                        ps2[:, m : m + 1],
                        w2sb[:, k * 512 + 128 * m : k * 512 + 128 * (m + 1)],
                        h1cols[:, k : k + 1],
                        start=(k == 0),
                        stop=(k == 1),
                    )
            p2sb = ap.tile([128, 4], F32)
            nc.vector.tensor_copy(p2sb[:], ps2[:])

            # ---- h2 exchange: single round of 7 single-dest broadcasts.
            # rcv2 is NEVER written locally: peer data may arrive before our
            # own instructions run (cross-core start skew). Sum is
            # slot-order-agnostic, so the exact delta->core map is irrelevant.
            rcv2 = cp.tile([128, 7 * 4], F32)
            for i, d in enumerate(range(1, 8)):
                rd = [None] * 8
                rd[d] = (0, d)
                nc.gpsimd.remote_dma_broadcast(
                    rcv2[:, 4 * i : 4 * (i + 1)], p2sb[:], rsem2, lsem,
                    rdests=rd, queue_num=1,
                )
            t2 = nc.gpsimd.trigger_dma(count=None, queue_num=1)
            mk2 = nc.vector.nop(nofuse=True, hint="rsem2_w")
            add_dep_helper(mk2.ins, t2.ins, sync=False, reason="wait after trigger")
            markers["h2"] = (mk2.ins.name, rsem2, 14)
            h2pre = ap.tile([128, 4], F32)
            s2 = nc.vector.tensor_tensor(h2pre[:], p2sb[:], rcv2[:, 0:4], ADD)
            add_dep_helper(s2.ins, mk2.ins, sync=False, reason="sum after sem wait")
            for i in range(1, 7):
                nc.vector.tensor_tensor(
                    h2pre[:], h2pre[:], rcv2[:, 4 * i : 4 * (i + 1)], ADD
                )
            h2cols = ap.tile([128, 4], BF16)
            for m in range(4):
                nc.scalar.activation(
                    h2cols[:, m : m + 1], h2pre[:, m : m + 1], Tanh,
                    bias=b2sb[:, m : m + 1],
                )

            # ---- L3: z = h2 @ W3 + b3 (no tanh) ----
            pz = psA.tile([128, 1], F32, tag="psA", name="pz")
            for k in range(4):
                nc.tensor.matmul(
                    pz[:], w3sb[:, 128 * k : 128 * (k + 1)], h2cols[:, k : k + 1],
                    start=(k == 0), stop=(k == 3),
                )
            zcol = ap.tile([128, 1], BF16)
            nc.scalar.activation(
                zcol[:], pz[:], mybir.ActivationFunctionType.Identity, bias=b3sb[:]
            )

            # ---- L4: h4 = tanh(z @ W4 + b4) ----
            ps4 = psA.tile([128, 4], F32, tag="psA", name="ps4")
            for m in range(4):
                nc.tensor.matmul(
                    ps4[:, m : m + 1], w4sb[:, 128 * m : 128 * (m + 1)], zcol[:],
                    start=True, stop=True,
                )
            h4cols = ap.tile([128, 4], BF16)
            for m in range(4):
                nc.scalar.activation(
                    h4cols[:, m : m + 1], ps4[:, m : m + 1], Tanh,
                    bias=b4sb[:, m : m + 1],
                )

            # ---- L5: h5 shard [128, 2] f32 (exchanged raw) ----
            ps5 = psA.tile([128, 2], F32, tag="psA", name="ps5")
            for m in range(2):
                for k in range(4):
                    nc.tensor.matmul(
                        ps5[:, m : m + 1],
                        w5sb[:, k * 256 + 128 * m : k * 256 + 128 * (m + 1)],
                        h4cols[:, k : k + 1],
                        start=(k == 0),
                        stop=(k == 3),
                    )
            h5colsb = ap.tile([128, 2], F32)
            for m in range(2):
                nc.scalar.activation(
                    h5colsb[:, m : m + 1], ps5[:, m : m + 1], Tanh,
                    bias=b5sb[:, m : m + 1],
                )

            # ---- h5 all-gather: own shard -> slot 0 locally; 7 broadcasts
            # deliver peer shards into slots d (delta d). W6 rows are
            # host-permuted to sigma(c, d) = BASE[BASE[c]^d] arrival order.
            rcv5 = cp.tile([128, 16], F32)
            nc.vector.tensor_copy(rcv5[:, 0:2], h5colsb[:])
            for d in range(1, 8):
                rd = [None] * 8
                rd[d] = (0, d)
                pr = nc.gpsimd.remote_dma_broadcast(
                    rcv5[:, 2 * d : 2 * (d + 1)], h5colsb[:], rsem5, lsem,
                    rdests=rd, queue_num=1,
                )
                add_dep_helper(pr.ins, t2.ins, sync=False, reason="after trig2")
            t5 = nc.gpsimd.trigger_dma(count=None, queue_num=1)
            mk5 = nc.vector.nop(nofuse=True, hint="rsem5_w")
            add_dep_helper(mk5.ins, t5.ins, sync=False, reason="wait after trigger")
            markers["h5"] = (mk5.ins.name, rsem5, 14)
            rcv5c = ap.tile([128, 16], BF16)
            c5 = nc.vector.tensor_copy(rcv5c[:], rcv5[:])
            add_dep_helper(c5.ins, mk5.ins, sync=False, reason="cast after sem wait")

            # ---- L6: y on own haloed span, weight-stationary fp8 FWL.
            # ps6[:, q] = y[2048c - 128 + 128q + p], exact (full 2048-sum).
            # W6 chunks are q-major (3 span-chunks each) so every column's
            # 16-matmul chain is contiguous (see L1 note) AND chunk-streamable.
            ps6 = ps6p.tile([128, NQ], F32, tag="ps6", name="ps6")
            for g in range(6):
                for ql in range(3):
                    q = 3 * g + ql
                    for kc in range(KC6):
                        nc.tensor.matmul(
                            ps6[:, q : q + 1],
                            w6sb[g][:, (ql * KC6 + kc) * 128 : (ql * KC6 + kc + 1) * 128],
                            rcv5c[:, kc : kc + 1],
                            start=(kc == 0),
                            stop=(kc == KC6 - 1),
                        )
            yv = ap.tile([128, NQ], F32)
            nc.vector.tensor_tensor(yv[:], ps6[:], b6sb[:], ADD)

            # ---- span -> [18, 128] chunk rows, then the windowed [18, 256]
            ptY = psA.tile([NQ, 128], F32, tag="psA", name="ptY")
            nc.tensor.transpose(ptY[:], yv[:], eyesb[:])
            red = ap.tile([NQ, 128], F32)
            nc.vector.tensor_copy(red[:], ptY[:])
            hf = ap.tile([NQ, 256], F32)
            nc.vector.memset(hf[0:1, 0:128], 0.0)
            nc.vector.tensor_copy(hf[:, 128:256], red[:])
            nc.sync.dma_start(hf[1:NQ, 0:128], red[0 : NQ - 1, :])

            hfe = ap.tile([NQ, 256], F32)
            nc.scalar.activation(hfe[:], hf[:], Exp, scale=1.0 / FP8_SCALE)
            sf = ap.tile([NQ, 256], F32)
            nc.vector.tensor_tensor_scan(sf[:], mf[:], hfe[:], 0.0, MUL, ADD)

            e_ap = hfe[:, 128:256]
            pt1 = psA.tile([128, NQ], F32, tag="psA", name="pt1")
            nc.tensor.transpose(pt1[:], e_ap, j18sb[:])
            ct1 = ap.tile([128, NQ], F32)
            nc.vector.tensor_copy(ct1[:], pt1[:])
            pt2 = psA.tile([NQ, 128], F32, tag="psA", name="pt2")
            nc.tensor.transpose(pt2[:], ct1[:], jsb[:])
            er = ap.tile([NQ, 128], F32)
            nc.vector.tensor_copy(er[:], pt2[:])
            psh = psA.tile([NQ, 128], F32, tag="psA", name="psh")
            nc.tensor.matmul(psh[:], sh18sb[:], er[:], start=True, stop=True)
            sr1 = ap.tile([NQ, 128], F32)
            nc.vector.tensor_tensor_scan(sr1[:], mr[:, 0:128], psh[:], 0.0, MUL, ADD)
            sr = ap.tile([NQ, 128], F32)
            nc.vector.tensor_tensor_scan(
                sr[:], mr[:, 128:256], er[:], sr1[:, 127:128], MUL, ADD
            )
            pt3 = psA.tile([128, NQ], F32, tag="psA", name="pt3")
            nc.tensor.transpose(pt3[:], sr[:], j18sb[:])
            ct3 = ap.tile([128, NQ], F32)
            nc.vector.tensor_copy(ct3[:], pt3[:])
            pt4 = psA.tile([NQ, 128], F32, tag="psA", name="pt4")
            nc.tensor.transpose(pt4[:], ct3[:], jsb[:])
            dd = ap.tile([NQ, 128], F32)
            nc.vector.tensor_tensor(dd[:], sf[:, 128:256], pt4[:], ADD)
            nc.vector.tensor_tensor(dd[:], dd[:], e_ap, SUB)
            rr = ap.tile([NQ, 128], F32)
            nc.vector.reciprocal(rr[:], dd[:])
            outt = ap.tile([NQ, 128], F32)
            nc.vector.tensor_tensor(outt[:], e_ap, rr[:], MUL)
            nc.gpsimd.dma_start(
                out_ext[:].rearrange("(a b) -> a b", b=128), outt[1 : NQ - 1, :]
            )

    # inject remote-sem waits on the marker nops (invisible to Tile's sim)
    want = {v[0]: (v[1], v[2]) for v in markers.values()}
    found = 0
    for f in nc.m.functions:
        for b in f.blocks:
            for inst in b.instructions:
                if inst.name in want:
                    sem, val = want[inst.name]
                    bass.BassInstruction(inst)._wait_ge(sem, val)
                    found += 1
    assert found == len(want), f"injected {found} of {len(want)} sem waits"
    _split_sync_waits(nc)
    lower_extended_insts(nc)
    return nc


def _prep_inputs(x, W1, b1, W2, b2, W3, b3, W4, b4, W5, b5, W6, b6, segment_ids):
    """Host-side sharding + layout permutation. Returns in_maps (one per core)."""
    x = np.ascontiguousarray(x, np.float32)
    seg = np.asarray(segment_ids)

    start = np.ones(DS, bool)
    start[1:] = seg[1:] != seg[:-1]
    end = np.ones(DS, bool)
    end[:-1] = seg[:-1] != seg[1:]
    seg_len = np.diff(np.concatenate([np.where(start)[0], [DS]]))
    assert seg_len.max() <= 128, f"segment too long for halo scan: {seg_len.max()}"

    eye = np.eye(128, dtype=np.float32)
    jr18 = np.eye(NQ, dtype=np.float32)[::-1].copy()
    jrev = eye[::-1].copy()
    shm18 = np.zeros((NQ, NQ), np.float32)
    shm18[np.arange(NQ - 1), np.arange(1, NQ)] = 1.0

    xTh = np.ascontiguousarray(x.reshape(128, 128).T).astype(mybir.dt.np(BF16))

    b2cv = np.ascontiguousarray(np.asarray(b2, np.float32).reshape(4, 128).T)
    b3cv = np.ascontiguousarray(np.asarray(b3, np.float32).reshape(1, 128).T)
    b4cv = np.ascontiguousarray(np.asarray(b4, np.float32).reshape(4, 128).T)

    W1 = np.asarray(W1, np.float32)
    W2 = np.asarray(W2, np.float32)
    W3 = np.asarray(W3, np.float32)
    W4 = np.asarray(W4, np.float32)
    W5 = np.asarray(W5, np.float32)
    W6 = np.asarray(W6, np.float32)
    b1 = np.asarray(b1, np.float32)
    b5 = np.asarray(b5, np.float32)
    b6 = np.asarray(b6, np.float32)

    w3h = np.ascontiguousarray(
        W3.reshape(4, 128, H3).transpose(1, 0, 2).reshape(128, 4 * H3)
    ).astype(mybir.dt.np(BF16))
    w4h = np.ascontiguousarray(W4).astype(mybir.dt.np(BF16))

    in_maps = []
    for c in range(NC):
        # L1 weight-stationary layout, m-major: chunk ct covers the
        # (m = ct//2, kc = 64*(ct%2) + kl) quarter; col = kl*128 + p
        w1s = W1[:, C1 * c : C1 * (c + 1)] * FP8_SCALE   # [16384, 256]
        w1h = np.ascontiguousarray(
            w1s.reshape(2, 64, 128, 2, 128)   # [cthalf, kl, k, m, p]
            .transpose(3, 0, 2, 1, 4)         # [m, cthalf, k, kl, p]
            .reshape(4, 128, 64 * 128)
        ).astype(mybir.dt.np(FP8))
        w2s = W2[C1 * c : C1 * (c + 1), :]
        w2h = np.ascontiguousarray(
            w2s.reshape(2, 128, H2).transpose(1, 0, 2).reshape(128, 2 * H2)
        ).astype(mybir.dt.np(BF16))
        w5s = W5[:, C1 * c : C1 * (c + 1)]
        w5h = np.ascontiguousarray(
            w5s.reshape(4, 128, C1).transpose(1, 0, 2).reshape(128, 4 * C1)
        ).astype(mybir.dt.np(BF16))

        # L6: column-shard = own haloed span; rows permuted to XOR arrival
        # order: rcv5 col (2d + m) holds h5[256*sigma(d) + 128m + k].
        sigma = [BASE[BASE[c] ^ d] for d in range(NC)]
        cols = (np.arange(SLICE * c - HALO, SLICE * (c + 1) + HALO)) % DS
        w6span = W6[:, cols] * FP8_SCALE                  # [2048, 2304]
        row_order = np.concatenate(
            [np.arange(C1 * s, C1 * (s + 1)) for s in sigma]
        )
        w6perm = w6span[row_order]                        # [16*128, 2304]
        w6h = np.ascontiguousarray(
            w6perm.reshape(KC6, 128, 6, 3, 128)           # [kc, k, g, ql, p]
            .transpose(2, 1, 3, 0, 4)                     # [g, k, ql, kc, p]
            .reshape(6, 128, 3 * KC6 * 128)
        ).astype(mybir.dt.np(FP8))
        b6qv = np.ascontiguousarray(
            (b6[cols] * FP8_SCALE).reshape(NQ, 128).T     # [p, q]
        )

        # per-core segmented-softmax masks over the haloed span of slice c
        st = start[cols].reshape(NQ, 128)
        en = end[cols].reshape(NQ, 128)
        m_own = (~st).astype(np.float32)
        mfh = np.zeros((NQ, 256), np.float32)
        mfh[1:, 0:128] = m_own[0 : NQ - 1]
        mfh[:, 128:256] = m_own
        m_rot = (~en).astype(np.float32)[::-1, ::-1]
        mrh = np.zeros((NQ, 256), np.float32)
        mrh[1:, 0:128] = m_rot[0 : NQ - 1]
        mrh[:, 128:256] = m_rot

        b1s = b1[C1 * c : C1 * (c + 1)]
        b5s = b5[C1 * c : C1 * (c + 1)]
        in_maps.append(
            {
                "xT": xTh,
                "w1": w1h,
                "w2": w2h,
                "w3": w3h,
                "w4": w4h,
                "w5": w5h,
                "w6": w6h,
                "b1c": np.ascontiguousarray(b1s.reshape(2, 128).T),
                "b2c": b2cv,
                "b3c": b3cv,
                "b4c": b4cv,
                "b5c": np.ascontiguousarray(b5s.reshape(2, 128).T),
                "b6q": b6qv,
                "eye": eye,
                "jrev": jrev,
                "jr18": jr18,
                "shm18": shm18,
                "mf": mfh,
                "mr": mrh,
            }
        )
    return in_maps


_GRAPH_CACHE = {}


def _get_graph():
    if "nc" not in _GRAPH_CACHE:
        _GRAPH_CACHE["nc"] = build_graph()
    return _GRAPH_CACHE["nc"]


def kernel(**inputs) -> np.ndarray:
    in_maps = _prep_inputs(**inputs)
    nc = _get_graph()
    res = run_bass_kernel_spmd(nc, in_maps, core_ids=list(range(NC)))
    return np.concatenate(
        [np.asarray(res.results[c]["out"], np.float32) for c in range(NC)]
    )


# revision 10
# speedup vs baseline: 1.0196x; 1.0196x over previous
"""Trainium2 Bass kernel for nn_AutoEncoder (segment_reduce).

6-layer MLP autoencoder on a single 16384-vector + segmented softmax over
1024 contiguous segments, distributed over 8 NeuronCores.

v4 design (vs the v2 CC-AllReduce baseline at ~161us):
  * All mat-vec layers run WEIGHT-STATIONARY: lhsT = 128x128 fp8/bf16
    weight block (Fast Weight Load eligible), rhs = the activation column
    [128, 1]. A weight-load-dominated fp8 matmul is ~40ns vs ~N cycles
    when the weights stream as the moving operand (the v2 scheme cost
    ~183ns per 256 streamed columns). L1 = 256 matmuls, L6 = 288.
  * NO y AllReduce (was 27us of CC time + 23us of DRAM-readback tail).
    Instead the tiny h5 vector (2048 f32) is all-gathered P2P: each core
    broadcasts its 256-value shard ([128, 2] f32, 1KB) to all 7 peers,
    then computes y EXACTLY on its own haloed 2304-column span of W6
    (column-sharded decoder). W6 rows per core are host-permuted to the
    XOR-delta arrival order sigma(c, d) = BASE[BASE[c]^d], so the shared
    SPMD instruction stream indexes receive slots uniformly.
  * h2 partial exchange collapsed from a 2-round XOR tree to a single
    round of 7 broadcasts (one latency, 14KB on the wire).
  * x is pre-transposed on the host ([128, 128] bf16 column layout), so
    no on-device PE transpose for it.
  * Softmax runs straight from SBUF ([128, 18] -> PE transpose -> the
    proven windowed-scan pipeline); output is each core's own 2048-slice.

Cross-box safety: logical delivery is D(j,d) = BASE[BASE[j]^d] with
BASE = (0,1,2,3,6,7,4,5). Ultra-pod die-flips XOR the whole real-NC table
by a constant, which cancels in D — verified empirically on this box.

Sem-wait handling: Tile's scheduling sim cannot model waits satisfied only
by peers, so remote-sem waits are injected post-Tile onto marker NoOps.
Receive slots are never written locally (no memset) to avoid racing
early-arriving peer data.
"""

import sys

if "/opt/trn_rl_repo" not in sys.path:
    sys.path.insert(0, "/opt/trn_rl_repo")

import numpy as np

import concourse.bass as bass
import concourse.mybir as mybir
import concourse.tile as tile
from concourse import library_config
from concourse.bass_utils import run_bass_kernel_spmd
from concourse.library_overlay import lower_extended_insts
from concourse.tile_rust import add_dep_helper

DS = 16384
H1, H2, H3 = 2048, 512, 128
NC = 8
C1 = H1 // NC       # 256  h1 / h5 shard
SLICE = DS // NC    # 2048 y-slice per core
HALO = 128
SPAN = SLICE + 2 * HALO          # 2304
NQ = SPAN // 128                 # 18 chunks of 128
KC1 = DS // 128                  # 128 k-chunks for L1
KC6 = H1 // 128                  # 16 k-chunks for L6
F32 = mybir.dt.float32
BF16 = mybir.dt.bfloat16
FP8 = mybir.dt.float8e4
FP8_SCALE = 64.0
BASE = (0, 1, 2, 3, 6, 7, 4, 5)  # logical -> real NC (global XOR offsets cancel)


def _split_sync_waits(nc):
    """The walrus build in this env only allows 1 sync wait on CTRL-class
    instructions (Drain/NoOp). Tile's tail drain carries one wait per live
    semaphore lane. Split excess waits onto preceding single-wait NOPs."""
    for f in nc.m.functions:
        for b in f.blocks:
            new_insts = []
            for inst in b.instructions:
                si = inst.sync_info
                if si is not None and si.on_wait and len(si.on_wait) > 1:
                    waits = list(si.on_wait)
                    head, tail = waits[:-1], waits[-1:]
                    for i, w in enumerate(head):
                        new_insts.append(
                            mybir.InstNoOp(
                                name=f"{inst.name}-ws{i}",
                                engine=inst.engine,
                                bass_nofuse=True,
                                sync_info=mybir.SyncInfo(on_wait=[w], on_update=[]),
                            )
                        )
                    si.on_wait = tail
                new_insts.append(inst)
            b.instructions = new_insts


def build_graph():
    nc = bass.Bass(num_swdge_queues=2)
    P = nc.declare_dram_parameter
    xT_in = P("xT", [128, 128], BF16, isOutput=False)       # xT[k, kc] = x[128kc+k]
    w1 = P("w1", [4, 128, 64 * 128], FP8, isOutput=False)   # m-major: (ct, k, kl*128+p)
    w2 = P("w2", [128, 2 * 512], BF16, isOutput=False)
    w3 = P("w3", [128, 4 * 128], BF16, isOutput=False)
    w4 = P("w4", [128, 512], BF16, isOutput=False)
    w5 = P("w5", [128, 4 * C1], BF16, isOutput=False)
    w6 = P("w6", [6, 128, 3 * KC6 * 128], FP8, isOutput=False)  # (g, k, (ql*16+kc)*128+p)
    b1c = P("b1c", [128, 2], F32, isOutput=False)
    b2c = P("b2c", [128, 4], F32, isOutput=False)
    b3c = P("b3c", [128, 1], F32, isOutput=False)
    b4c = P("b4c", [128, 4], F32, isOutput=False)
    b5c = P("b5c", [128, 2], F32, isOutput=False)
    b6q = P("b6q", [128, NQ], F32, isOutput=False)          # b6 span cols * scale
    eye = P("eye", [128, 128], F32, isOutput=False)
    jrev = P("jrev", [128, 128], F32, isOutput=False)
    jr18 = P("jr18", [NQ, NQ], F32, isOutput=False)
    shm18 = P("shm18", [NQ, NQ], F32, isOutput=False)
    mf_in = P("mf", [NQ, 256], F32, isOutput=False)
    mr_in = P("mr", [NQ, 256], F32, isOutput=False)
    out_ext = P("out", [SLICE], F32, isOutput=True)

    Tanh = mybir.ActivationFunctionType.Tanh
    Exp = mybir.ActivationFunctionType.Exp
    ADD = mybir.AluOpType.add
    SUB = mybir.AluOpType.subtract
    MUL = mybir.AluOpType.mult

    rsem2 = nc.alloc_semaphore("rsem2")
    rsem5 = nc.alloc_semaphore("rsem5")
    lsem = nc.alloc_semaphore("lsem_rdma")
    markers = {}

    with tile.TileContext(nc) as tc:
        with (
            tc.tile_pool(name="const", bufs=1) as cp,
            tc.tile_pool(name="w1p", bufs=4) as w1p,
            tc.tile_pool(name="w6p", bufs=6) as w6p,
            tc.tile_pool(name="act", bufs=1) as ap,
            tc.tile_pool(name="psA", bufs=2, space="PSUM") as psA,
            tc.tile_pool(name="ps1p", bufs=1, space="PSUM") as ps1p,
            tc.tile_pool(name="ps6p", bufs=1, space="PSUM") as ps6p,
            tc.tile_pool(name="dram", bufs=1, space="DRAM") as dp,
        ):
            # gpsimd: load the remote_dma ucode library up front (SWDGE queue
            # otherwise idle until the exchanges)
            nc.gpsimd.load_library(library_config.remote_dma)

            # Dummy CC AllGather: its presence in the NEFF forces a
            # synchronized cross-rank launch (without any CC instruction the
            # axon runtime dispatches cores milliseconds apart, which any
            # cross-core exchange then serializes on). Nothing consumes the
            # result; it also absorbs ncfw's first-collective wake latency.
            dumin = dp.tile([8], F32)
            dumout = dp.tile([8 * NC], F32, addr_space="Shared")
            nc.gpsimd.collective_compute(
                "AllGather", mybir.AluOpType.bypass,
                ins=[dumin[:].opt()], outs=[dumout[:].opt()],
                replica_groups=[list(range(NC))],
            )

            # ---- sync HWDGE ring: x first (needed earliest), then encoder
            xT = cp.tile([128, 128], BF16)
            nc.sync.dma_start(xT[:], xT_in[:])
            eyesb = cp.tile([128, 128], F32)
            nc.sync.dma_start(eyesb[:], eye[:])
            b1sb = cp.tile([128, 2], F32)
            nc.sync.dma_start(b1sb[:], b1c[:])
            w2sb = cp.tile([128, 1024], BF16)
            nc.sync.dma_start(w2sb[:], w2[:])
            w3sb = cp.tile([128, 512], BF16)
            nc.sync.dma_start(w3sb[:], w3[:])
            b2sb = cp.tile([128, 4], F32)
            nc.sync.dma_start(b2sb[:], b2c[:])
            b3sb = cp.tile([128, 1], F32)
            nc.sync.dma_start(b3sb[:], b3c[:])
            # ---- scalar ring: softmax constants + later-needed weights
            jsb = cp.tile([128, 128], F32)
            nc.scalar.dma_start(jsb[:], jrev[:])
            j18sb = cp.tile([NQ, NQ], F32)
            nc.scalar.dma_start(j18sb[:], jr18[:])
            sh18sb = cp.tile([NQ, NQ], F32)
            nc.scalar.dma_start(sh18sb[:], shm18[:])
            mf = cp.tile([NQ, 256], F32)
            nc.scalar.dma_start(mf[:], mf_in[:])
            mr = cp.tile([NQ, 256], F32)
            nc.scalar.dma_start(mr[:], mr_in[:])
            w4sb = cp.tile([128, 512], BF16)
            nc.scalar.dma_start(w4sb[:], w4[:])
            w5sb = cp.tile([128, 1024], BF16)
            nc.scalar.dma_start(w5sb[:], w5[:])
            b4sb = cp.tile([128, 4], F32)
            nc.scalar.dma_start(b4sb[:], b4c[:])
            b5sb = cp.tile([128, 2], F32)
            nc.scalar.dma_start(b5sb[:], b5c[:])
            b6sb = cp.tile([128, NQ], F32)
            nc.scalar.dma_start(b6sb[:], b6q[:])

            # ---- bulk W1 + W6 stream on SWDGE queue 0 (fast desc-gen);
            # the exchanges ride SWDGE queue 1 so they never queue behind bulk
            w1sb_l = []
            for ct in range(4):
                t = w1p.tile([128, 64 * 128], FP8, tag="w1sb", name=f"w1sb{ct}")
                nc.gpsimd.dma_start(t[:], w1[ct])
                w1sb_l.append(t)
            w6sb = []
            for g in range(6):
                t = w6p.tile([128, 3 * KC6 * 128], FP8, tag="w6", name=f"w6sb{g}")
                nc.gpsimd.dma_start(t[:], w6[g])
                w6sb.append(t)

            # ---- L1: h1 shard [128, 2] cols, weight-stationary fp8 FWL.
            # CRITICAL: each PSUM column's accumulation chain must be
            # CONTIGUOUS — interleaving two open chains on the PE corrupts
            # the accumulation (~6e-2 rel err, micro-verified). W1 is laid
            # out m-major so chains stay chunk-streamable.
            ps1 = ps1p.tile([128, 2], F32, tag="ps1", name="ps1")
            for ct in range(4):
                m = ct // 2
                for kl in range(64):
                    kc = 64 * (ct % 2) + kl
                    nc.tensor.matmul(
                        ps1[:, m : m + 1],
                        w1sb_l[ct][:, kl * 128 : (kl + 1) * 128],
                        xT[:, kc : kc + 1],
                        start=(ct % 2 == 0 and kl == 0),
                        stop=(ct % 2 == 1 and kl == 63),
                    )
            h1cols = ap.tile([128, 2], BF16)
            for m in range(2):
                nc.scalar.activation(
                    h1cols[:, m : m + 1], ps1[:, m : m + 1], Tanh,
                    bias=b1sb[:, m : m + 1], scale=1.0 / FP8_SCALE,
                )

            # ---- L2 partials ----
            ps2 = psA.tile([128, 4], F32, tag="psA", name="ps2")
            for m in range(4):
                for k in range(2):
                    nc.tensor.matmul(
                        ps2[:, m : m + 1],
                        w2sb[:, k * 512 + 128 * m : k * 512 + 128 * (m + 1)],
                        h1cols[:, k : k + 1],
                        start=(k == 0),
                        stop=(k == 1),
                    )
            p2sb = ap.tile([128, 4], F32)
            nc.vector.tensor_copy(p2sb[:], ps2[:])

            # ---- h2 exchange: single round of 7 single-dest broadcasts.
            # rcv2 is NEVER written locally: peer data may arrive before our
            # own instructions run (cross-core start skew). Sum is
            # slot-order-agnostic, so the exact delta->core map is irrelevant.
            rcv2 = cp.tile([128, 7 * 4], F32)
            for i, d in enumerate(range(1, 8)):
                rd = [None] * 8
                rd[d] = (0, d)
                nc.gpsimd.remote_dma_broadcast(
                    rcv2[:, 4 * i : 4 * (i + 1)], p2sb[:], rsem2, lsem,
                    rdests=rd, queue_num=1,
                )
            t2 = nc.gpsimd.trigger_dma(count=None, queue_num=1)
            mk2 = nc.vector.nop(nofuse=True, hint="rsem2_w")
            add_dep_helper(mk2.ins, t2.ins, sync=False, reason="wait after trigger")
            markers["h2"] = (mk2.ins.name, rsem2, 14)
            h2pre = ap.tile([128, 4], F32)
            s2 = nc.vector.tensor_tensor(h2pre[:], p2sb[:], rcv2[:, 0:4], ADD)
            add_dep_helper(s2.ins, mk2.ins, sync=False, reason="sum after sem wait")
            for i in range(1, 7):
                nc.vector.tensor_tensor(
                    h2pre[:], h2pre[:], rcv2[:, 4 * i : 4 * (i + 1)], ADD
                )
            h2cols = ap.tile([128, 4], BF16)
            for m in range(4):
                nc.scalar.activation(
                    h2cols[:, m : m + 1], h2pre[:, m : m + 1], Tanh,
                    bias=b2sb[:, m : m + 1],
                )

            # ---- L3: z = h2 @ W3 + b3 (no tanh) ----
            pz = psA.tile([128, 1], F32, tag="psA", name="pz")
            for k in range(4):
                nc.tensor.matmul(
                    pz[:], w3sb[:, 128 * k : 128 * (k + 1)], h2cols[:, k : k + 1],
                    start=(k == 0), stop=(k == 3),
                )
            zcol = ap.tile([128, 1], BF16)
            nc.scalar.activation(
                zcol[:], pz[:], mybir.ActivationFunctionType.Identity, bias=b3sb[:]
            )

            # ---- L4: h4 = tanh(z @ W4 + b4) ----
            ps4 = psA.tile([128, 4], F32, tag="psA", name="ps4")
            for m in range(4):
                nc.tensor.matmul(
                    ps4[:, m : m + 1], w4sb[:, 128 * m : 128 * (m + 1)], zcol[:],
                    start=True, stop=True,
                )
            h4cols = ap.tile([128, 4], BF16)
            for m in range(4):
                nc.scalar.activation(
                    h4cols[:, m : m + 1], ps4[:, m : m + 1], Tanh,
                    bias=b4sb[:, m : m + 1],
                )

            # ---- L5: h5 shard [128, 2] f32 (exchanged raw) ----
            ps5 = psA.tile([128, 2], F32, tag="psA", name="ps5")
            for m in range(2):
                for k in range(4):
                    nc.tensor.matmul(
                        ps5[:, m : m + 1],
                        w5sb[:, k * 256 + 128 * m : k * 256 + 128 * (m + 1)],
                        h4cols[:, k : k + 1],
                        start=(k == 0),
                        stop=(k == 3),
                    )
            h5colsb = ap.tile([128, 2], F32)
            for m in range(2):
                nc.scalar.activation(
                    h5colsb[:, m : m + 1], ps5[:, m : m + 1], Tanh,
                    bias=b5sb[:, m : m + 1],
                )

            # ---- h5 all-gather: own shard -> slot 0 locally; 7 broadcasts
            # deliver peer shards into slots d (delta d). W6 rows are
            # host-permuted to sigma(c, d) = BASE[BASE[c]^d] arrival order.
            rcv5 = cp.tile([128, 16], F32)
            nc.vector.tensor_copy(rcv5[:, 0:2], h5colsb[:])
            for d in range(1, 8):
                rd = [None] * 8
                rd[d] = (0, d)
                pr = nc.gpsimd.remote_dma_broadcast(
                    rcv5[:, 2 * d : 2 * (d + 1)], h5colsb[:], rsem5, lsem,
                    rdests=rd, queue_num=1,
                )
                add_dep_helper(pr.ins, t2.ins, sync=False, reason="after trig2")
            t5 = nc.gpsimd.trigger_dma(count=None, queue_num=1)
            mk5 = nc.vector.nop(nofuse=True, hint="rsem5_w")
            add_dep_helper(mk5.ins, t5.ins, sync=False, reason="wait after trigger")
            markers["h5"] = (mk5.ins.name, rsem5, 14)
            rcv5c = ap.tile([128, 16], BF16)
            c5 = nc.vector.tensor_copy(rcv5c[:], rcv5[:])
            add_dep_helper(c5.ins, mk5.ins, sync=False, reason="cast after sem wait")

            # ---- L6: y on own haloed span, weight-stationary fp8 FWL.
            # ps6[:, q] = y[2048c - 128 + 128q + p], exact (full 2048-sum).
            # W6 chunks are q-major (3 span-chunks each) so every column's
            # 16-matmul chain is contiguous (see L1 note) AND chunk-streamable.
            ps6 = ps6p.tile([128, NQ], F32, tag="ps6", name="ps6")
            for g in range(6):
                for ql in range(3):
                    q = 3 * g + ql
                    for kc in range(KC6):
                        nc.tensor.matmul(
                            ps6[:, q : q + 1],
                            w6sb[g][:, (ql * KC6 + kc) * 128 : (ql * KC6 + kc + 1) * 128],
                            rcv5c[:, kc : kc + 1],
                            start=(kc == 0),
                            stop=(kc == KC6 - 1),
                        )
            yv = ap.tile([128, NQ], F32)
            nc.vector.tensor_tensor(yv[:], ps6[:], b6sb[:], ADD)

            # ---- span -> [18, 128] chunk rows, then the windowed [18, 256]
            ptY = psA.tile([NQ, 128], F32, tag="psA", name="ptY")
            nc.tensor.transpose(ptY[:], yv[:], eyesb[:])
            red = ap.tile([NQ, 128], F32)
            nc.vector.tensor_copy(red[:], ptY[:])
            hf = ap.tile([NQ, 256], F32)
            nc.vector.memset(hf[0:1, 0:128], 0.0)
            nc.vector.tensor_copy(hf[:, 128:256], red[:])
            nc.sync.dma_start(hf[1:NQ, 0:128], red[0 : NQ - 1, :])

            hfe = ap.tile([NQ, 256], F32)
            nc.scalar.activation(hfe[:], hf[:], Exp, scale=1.0 / FP8_SCALE)
            sf = ap.tile([NQ, 256], F32)
            nc.vector.tensor_tensor_scan(sf[:], mf[:], hfe[:], 0.0, MUL, ADD)

            e_ap = hfe[:, 128:256]
            pt1 = psA.tile([128, NQ], F32, tag="psA", name="pt1")
            nc.tensor.transpose(pt1[:], e_ap, j18sb[:])
            ct1 = ap.tile([128, NQ], F32)
            nc.vector.tensor_copy(ct1[:], pt1[:])
            pt2 = psA.tile([NQ, 128], F32, tag="psA", name="pt2")
            nc.tensor.transpose(pt2[:], ct1[:], jsb[:])
            er = ap.tile([NQ, 128], F32)
            nc.vector.tensor_copy(er[:], pt2[:])
            psh = psA.tile([NQ, 128], F32, tag="psA", name="psh")
            nc.tensor.matmul(psh[:], sh18sb[:], er[:], start=True, stop=True)
            sr1 = ap.tile([NQ, 128], F32)
            nc.vector.tensor_tensor_scan(sr1[:], mr[:, 0:128], psh[:], 0.0, MUL, ADD)
            sr = ap.tile([NQ, 128], F32)
            nc.vector.tensor_tensor_scan(
                sr[:], mr[:, 128:256], er[:], sr1[:, 127:128], MUL, ADD
            )
            pt3 = psA.tile([128, NQ], F32, tag="psA", name="pt3")
            nc.tensor.transpose(pt3[:], sr[:], j18sb[:])
            ct3 = ap.tile([128, NQ], F32)
            nc.vector.tensor_copy(ct3[:], pt3[:])
            pt4 = psA.tile([NQ, 128], F32, tag="psA", name="pt4")
            nc.tensor.transpose(pt4[:], ct3[:], jsb[:])
            dd = ap.tile([NQ, 128], F32)
            nc.vector.tensor_tensor(dd[:], sf[:, 128:256], pt4[:], ADD)
            nc.vector.tensor_tensor(dd[:], dd[:], e_ap, SUB)
            rr = ap.tile([NQ, 128], F32)
            nc.vector.reciprocal(rr[:], dd[:])
            outt = ap.tile([NQ, 128], F32)
            nc.vector.tensor_tensor(outt[:], e_ap, rr[:], MUL)
            nc.gpsimd.dma_start(
                out_ext[:].rearrange("(a b) -> a b", b=128), outt[1 : NQ - 1, :]
            )

    # Strip Tile-inserted waits from the dummy AllGather: Tile schedules it
    # after all input DMAs, so each rank would join the collective only after
    # its full weight stream (~45us) and the tail drain then waits for the
    # slowest rank's join (~skew + 45us), gating the NEFF end. With no waits
    # every rank joins at launch; it completes at ~skew and gates nothing.
    for f in nc.m.functions:
        for b in f.blocks:
            for inst in b.instructions:
                if isinstance(inst, mybir.InstCollectiveCompute):
                    si = inst.sync_info
                    if si is not None:
                        si.on_wait = []

    # inject remote-sem waits on the marker nops (invisible to Tile's sim)
    want = {v[0]: (v[1], v[2]) for v in markers.values()}
    found = 0
    for f in nc.m.functions:
        for b in f.blocks:
            for inst in b.instructions:
                if inst.name in want:
                    sem, val = want[inst.name]
                    bass.BassInstruction(inst)._wait_ge(sem, val)
                    found += 1
    assert found == len(want), f"injected {found} of {len(want)} sem waits"
    _split_sync_waits(nc)
    lower_extended_insts(nc)
    return nc


def _prep_inputs(x, W1, b1, W2, b2, W3, b3, W4, b4, W5, b5, W6, b6, segment_ids):
    """Host-side sharding + layout permutation. Returns in_maps (one per core)."""
    x = np.ascontiguousarray(x, np.float32)
    seg = np.asarray(segment_ids)

    start = np.ones(DS, bool)
    start[1:] = seg[1:] != seg[:-1]
    end = np.ones(DS, bool)
    end[:-1] = seg[:-1] != seg[1:]
    seg_len = np.diff(np.concatenate([np.where(start)[0], [DS]]))
    assert seg_len.max() <= 128, f"segment too long for halo scan: {seg_len.max()}"

    eye = np.eye(128, dtype=np.float32)
    jr18 = np.eye(NQ, dtype=np.float32)[::-1].copy()
    jrev = eye[::-1].copy()
    shm18 = np.zeros((NQ, NQ), np.float32)
    shm18[np.arange(NQ - 1), np.arange(1, NQ)] = 1.0

    xTh = np.ascontiguousarray(x.reshape(128, 128).T).astype(mybir.dt.np(BF16))

    b2cv = np.ascontiguousarray(np.asarray(b2, np.float32).reshape(4, 128).T)
    b3cv = np.ascontiguousarray(np.asarray(b3, np.float32).reshape(1, 128).T)
    b4cv = np.ascontiguousarray(np.asarray(b4, np.float32).reshape(4, 128).T)

    W1 = np.asarray(W1, np.float32)
    W2 = np.asarray(W2, np.float32)
    W3 = np.asarray(W3, np.float32)
    W4 = np.asarray(W4, np.float32)
    W5 = np.asarray(W5, np.float32)
    W6 = np.asarray(W6, np.float32)
    b1 = np.asarray(b1, np.float32)
    b5 = np.asarray(b5, np.float32)
    b6 = np.asarray(b6, np.float32)

    w3h = np.ascontiguousarray(
        W3.reshape(4, 128, H3).transpose(1, 0, 2).reshape(128, 4 * H3)
    ).astype(mybir.dt.np(BF16))
    w4h = np.ascontiguousarray(W4).astype(mybir.dt.np(BF16))

    in_maps = []
    for c in range(NC):
        # L1 weight-stationary layout, m-major: chunk ct covers the
        # (m = ct//2, kc = 64*(ct%2) + kl) quarter; col = kl*128 + p
        w1s = W1[:, C1 * c : C1 * (c + 1)] * FP8_SCALE   # [16384, 256]
        w1h = np.ascontiguousarray(
            w1s.reshape(2, 64, 128, 2, 128)   # [cthalf, kl, k, m, p]
            .transpose(3, 0, 2, 1, 4)         # [m, cthalf, k, kl, p]
            .reshape(4, 128, 64 * 128)
        ).astype(mybir.dt.np(FP8))
        w2s = W2[C1 * c : C1 * (c + 1), :]
        w2h = np.ascontiguousarray(
            w2s.reshape(2, 128, H2).transpose(1, 0, 2).reshape(128, 2 * H2)
        ).astype(mybir.dt.np(BF16))
        w5s = W5[:, C1 * c : C1 * (c + 1)]
        w5h = np.ascontiguousarray(
            w5s.reshape(4, 128, C1).transpose(1, 0, 2).reshape(128, 4 * C1)
        ).astype(mybir.dt.np(BF16))

        # L6: column-shard = own haloed span; rows permuted to XOR arrival
        # order: rcv5 col (2d + m) holds h5[256*sigma(d) + 128m + k].
        sigma = [BASE[BASE[c] ^ d] for d in range(NC)]
        cols = (np.arange(SLICE * c - HALO, SLICE * (c + 1) + HALO)) % DS
        w6span = W6[:, cols] * FP8_SCALE                  # [2048, 2304]
        row_order = np.concatenate(
            [np.arange(C1 * s, C1 * (s + 1)) for s in sigma]
        )
        w6perm = w6span[row_order]                        # [16*128, 2304]
        w6h = np.ascontiguousarray(
            w6perm.reshape(KC6, 128, 6, 3, 128)           # [kc, k, g, ql, p]
            .transpose(2, 1, 3, 0, 4)                     # [g, k, ql, kc, p]
            .reshape(6, 128, 3 * KC6 * 128)
        ).astype(mybir.dt.np(FP8))
        b6qv = np.ascontiguousarray(
            (b6[cols] * FP8_SCALE).reshape(NQ, 128).T     # [p, q]
        )

        # per-core segmented-softmax masks over the haloed span of slice c
        st = start[cols].reshape(NQ, 128)
        en = end[cols].reshape(NQ, 128)
        m_own = (~st).astype(np.float32)
        mfh = np.zeros((NQ, 256), np.float32)
        mfh[1:, 0:128] = m_own[0 : NQ - 1]
        mfh[:, 128:256] = m_own
        m_rot = (~en).astype(np.float32)[::-1, ::-1]
        mrh = np.zeros((NQ, 256), np.float32)
        mrh[1:, 0:128] = m_rot[0 : NQ - 1]
        mrh[:, 128:256] = m_rot

        b1s = b1[C1 * c : C1 * (c + 1)]
        b5s = b5[C1 * c : C1 * (c + 1)]
        in_maps.append(
            {
                "xT": xTh,
                "w1": w1h,
                "w2": w2h,
                "w3": w3h,
                "w4": w4h,
                "w5": w5h,
                "w6": w6h,
                "b1c": np.ascontiguousarray(b1s.reshape(2, 128).T),
                "b2c": b2cv,
                "b3c": b3cv,
                "b4c": b4cv,
                "b5c": np.ascontiguousarray(b5s.reshape(2, 128).T),
                "b6q": b6qv,
                "eye": eye,
                "jrev": jrev,
                "jr18": jr18,
                "shm18": shm18,
                "mf": mfh,
                "mr": mrh,
            }
        )
    return in_maps


_GRAPH_CACHE = {}


def _get_graph():
    if "nc" not in _GRAPH_CACHE:
        _GRAPH_CACHE["nc"] = build_graph()
    return _GRAPH_CACHE["nc"]


def kernel(**inputs) -> np.ndarray:
    in_maps = _prep_inputs(**inputs)
    nc = _get_graph()
    res = run_bass_kernel_spmd(nc, in_maps, core_ids=list(range(NC)))
    return np.concatenate(
        [np.asarray(res.results[c]["out"], np.float32) for c in range(NC)]
    )


# revision 11
# speedup vs baseline: 1.5235x; 1.4942x over previous
"""Trainium2 Bass kernel for nn_AutoEncoder (segment_reduce).

6-layer MLP autoencoder on a single 16384-vector + segmented softmax over
1024 contiguous segments, distributed over 8 NeuronCores.

v5 design (vs the v2 CC-AllReduce baseline at ~161us):
  * All mat-vec layers run WEIGHT-STATIONARY: lhsT = 128x128 fp8 weight
    block (Fast Weight Load eligible), rhs = the activation column
    [128, 1]. A weight-load-dominated fp8 matmul is ~40ns vs N cycles
    when the weights stream as the moving operand. fp8-lhsT x bf16-rhs
    is numerically EXACT (micro-verified); but each PSUM column's
    accumulation chain must be CONTIGUOUS — interleaving two open chains
    corrupts the accumulation (~6e-2 rel err, micro-verified). Weight
    layouts are m-major / q-major so chains stay chunk-streamable.
  * ONE cross-core sync point: a CC AllGather of the h1 shard (256 f32,
    1KB). The middle layers W2..W5 are replicated (fp8, +2.2MB DMA,
    hidden under the W1/W6 stream), so h2..h5 are computed fully locally
    and the decoder needs no second exchange and no XOR-delta games.
    (P2P remote_dma was tried: each broadcast frame costs ~7us in the
    SWDGE/D2D path, so two 7-frame exchanges burned ~90us.)
  * Each core computes y exactly on its own haloed 2304-column span of
    W6 (column-sharded decoder) and runs the windowed segmented softmax
    from SBUF; output is the core's own 2048-slice.
  * A dummy CC AllGather with its Tile-inserted waits stripped runs at
    launch: it forces the synchronized cross-rank dispatch and absorbs
    ncfw's first-collective wake latency in parallel with the stream.
"""

import sys

if "/opt/trn_rl_repo" not in sys.path:
    sys.path.insert(0, "/opt/trn_rl_repo")

import numpy as np

import concourse.bass as bass
import concourse.mybir as mybir
import concourse.tile as tile
from concourse.bass_utils import run_bass_kernel_spmd
from concourse.library_overlay import lower_extended_insts

DS = 16384
H1, H2, H3 = 2048, 512, 128
NC = 8
C1 = H1 // NC       # 256  h1 shard
SLICE = DS // NC    # 2048 y-slice per core
HALO = 128
SPAN = SLICE + 2 * HALO          # 2304
NQ = SPAN // 128                 # 18 chunks of 128
KC6 = H1 // 128                  # 16 k-chunks for L6
F32 = mybir.dt.float32
BF16 = mybir.dt.bfloat16
FP8 = mybir.dt.float8e4
FP8_SCALE = 64.0


def _split_sync_waits(nc):
    """The walrus build in this env only allows 1 sync wait on CTRL-class
    instructions (Drain/NoOp). Tile's tail drain carries one wait per live
    semaphore lane. Split excess waits onto preceding single-wait NOPs."""
    for f in nc.m.functions:
        for b in f.blocks:
            new_insts = []
            for inst in b.instructions:
                si = inst.sync_info
                if si is not None and si.on_wait and len(si.on_wait) > 1:
                    waits = list(si.on_wait)
                    head, tail = waits[:-1], waits[-1:]
                    for i, w in enumerate(head):
                        new_insts.append(
                            mybir.InstNoOp(
                                name=f"{inst.name}-ws{i}",
                                engine=inst.engine,
                                bass_nofuse=True,
                                sync_info=mybir.SyncInfo(on_wait=[w], on_update=[]),
                            )
                        )
                    si.on_wait = tail
                new_insts.append(inst)
            b.instructions = new_insts


def build_graph():
    nc = bass.Bass(num_swdge_queues=1)
    P = nc.declare_dram_parameter
    xT_in = P("xT", [128, 128], BF16, isOutput=False)       # xT[k, kc] = x[128kc+k]
    w1 = P("w1", [4, 128, 64 * 128], FP8, isOutput=False)   # m-major: (ct, k, kl*128+p)
    w2 = P("w2", [128, 4 * 16 * 128], FP8, isOutput=False)  # (k, (m*16+kc)*128+p)
    w3 = P("w3", [128, 4 * 128], FP8, isOutput=False)       # (k, kc*128+p)
    w4 = P("w4", [128, 4 * 128], FP8, isOutput=False)       # (k, m*128+p)
    w5 = P("w5", [128, 16 * 4 * 128], FP8, isOutput=False)  # (k, (m*4+kc)*128+p)
    w6 = P("w6", [6, 128, 3 * KC6 * 128], FP8, isOutput=False)  # (g, k, (ql*16+kc)*128+p)
    b1c = P("b1c", [128, 2], F32, isOutput=False)
    b2c = P("b2c", [128, 4], F32, isOutput=False)
    b3c = P("b3c", [128, 1], F32, isOutput=False)
    b4c = P("b4c", [128, 4], F32, isOutput=False)
    b5c = P("b5c", [128, 16], F32, isOutput=False)
    b6q = P("b6q", [128, NQ], F32, isOutput=False)          # b6 span cols * scale
    eye = P("eye", [128, 128], F32, isOutput=False)
    jrev = P("jrev", [128, 128], F32, isOutput=False)
    jr18 = P("jr18", [NQ, NQ], F32, isOutput=False)
    shm18 = P("shm18", [NQ, NQ], F32, isOutput=False)
    mf_in = P("mf", [NQ, 256], F32, isOutput=False)
    mr_in = P("mr", [NQ, 256], F32, isOutput=False)
    out_ext = P("out", [SLICE], F32, isOutput=True)

    Tanh = mybir.ActivationFunctionType.Tanh
    Exp = mybir.ActivationFunctionType.Exp
    ADD = mybir.AluOpType.add
    SUB = mybir.AluOpType.subtract
    MUL = mybir.AluOpType.mult
    RG = [list(range(NC))]
    dummy_cc_names = []

    with tile.TileContext(nc) as tc:
        with (
            tc.tile_pool(name="const", bufs=1) as cp,
            tc.tile_pool(name="w1p", bufs=4) as w1p,
            tc.tile_pool(name="w6p", bufs=6) as w6p,
            tc.tile_pool(name="act", bufs=1) as ap,
            tc.tile_pool(name="psA", bufs=2, space="PSUM") as psA,
            tc.tile_pool(name="ps1p", bufs=1, space="PSUM") as ps1p,
            tc.tile_pool(name="ps5p", bufs=1, space="PSUM") as ps5p,
            tc.tile_pool(name="ps6p", bufs=1, space="PSUM") as ps6p,
            tc.tile_pool(name="dram", bufs=1, space="DRAM") as dp,
        ):
            # Dummy CC AllGather (waits stripped post-Tile): forces the
            # synchronized cross-rank launch and absorbs ncfw's
            # first-collective wake latency while the stream runs.
            dumin = dp.tile([8], F32)
            dumout = dp.tile([8 * NC], F32, addr_space="Shared")
            cc0 = nc.gpsimd.collective_compute(
                "AllGather", mybir.AluOpType.bypass,
                ins=[dumin[:].opt()], outs=[dumout[:].opt()],
                replica_groups=RG,
            )
            dummy_cc_names.append(cc0.ins.name)

            # ---- sync HWDGE ring: x first, then the replicated encoder tail
            xT = cp.tile([128, 128], BF16)
            nc.sync.dma_start(xT[:], xT_in[:])
            eyesb = cp.tile([128, 128], F32)
            nc.sync.dma_start(eyesb[:], eye[:])
            b1sb = cp.tile([128, 2], F32)
            nc.sync.dma_start(b1sb[:], b1c[:])
            w2sb = cp.tile([128, 8192], FP8)
            nc.sync.dma_start(w2sb[:], w2[:])
            w3sb = cp.tile([128, 512], FP8)
            nc.sync.dma_start(w3sb[:], w3[:])
            b2sb = cp.tile([128, 4], F32)
            nc.sync.dma_start(b2sb[:], b2c[:])
            b3sb = cp.tile([128, 1], F32)
            nc.sync.dma_start(b3sb[:], b3c[:])
            # ---- scalar ring: decoder weights + softmax constants
            w5sb = cp.tile([128, 8192], FP8)
            nc.scalar.dma_start(w5sb[:], w5[:])
            w4sb = cp.tile([128, 512], FP8)
            nc.scalar.dma_start(w4sb[:], w4[:])
            jsb = cp.tile([128, 128], F32)
            nc.scalar.dma_start(jsb[:], jrev[:])
            j18sb = cp.tile([NQ, NQ], F32)
            nc.scalar.dma_start(j18sb[:], jr18[:])
            sh18sb = cp.tile([NQ, NQ], F32)
            nc.scalar.dma_start(sh18sb[:], shm18[:])
            mf = cp.tile([NQ, 256], F32)
            nc.scalar.dma_start(mf[:], mf_in[:])
            mr = cp.tile([NQ, 256], F32)
            nc.scalar.dma_start(mr[:], mr_in[:])
            b4sb = cp.tile([128, 4], F32)
            nc.scalar.dma_start(b4sb[:], b4c[:])
            b5sb = cp.tile([128, 16], F32)
            nc.scalar.dma_start(b5sb[:], b5c[:])
            b6sb = cp.tile([128, NQ], F32)
            nc.scalar.dma_start(b6sb[:], b6q[:])

            # ---- bulk W1 + W6 stream on SWDGE queue 0
            w1sb_l = []
            for ct in range(4):
                t = w1p.tile([128, 64 * 128], FP8, tag="w1sb", name=f"w1sb{ct}")
                nc.gpsimd.dma_start(t[:], w1[ct])
                w1sb_l.append(t)
            w6sb = []
            for g in range(6):
                t = w6p.tile([128, 3 * KC6 * 128], FP8, tag="w6", name=f"w6sb{g}")
                nc.gpsimd.dma_start(t[:], w6[g])
                w6sb.append(t)

            # ---- L1: h1 shard [128, 2] cols; m-major contiguous chains
            ps1 = ps1p.tile([128, 2], F32, tag="ps1", name="ps1")
            for ct in range(4):
                m = ct // 2
                for kl in range(64):
                    kc = 64 * (ct % 2) + kl
                    nc.tensor.matmul(
                        ps1[:, m : m + 1],
                        w1sb_l[ct][:, kl * 128 : (kl + 1) * 128],
                        xT[:, kc : kc + 1],
                        start=(ct % 2 == 0 and kl == 0),
                        stop=(ct % 2 == 1 and kl == 63),
                    )
            h1c = ap.tile([128, 2], F32)
            for m in range(2):
                nc.scalar.activation(
                    h1c[:, m : m + 1], ps1[:, m : m + 1], Tanh,
                    bias=b1sb[:, m : m + 1], scale=1.0 / FP8_SCALE,
                )

            # ---- h1 all-gather via the CC engine (rank-ordered natively):
            # shard -> row layout -> DRAM -> AllGather -> SBUF -> columns
            psT1 = psA.tile([2, 128], F32, tag="psA", name="psT1")
            nc.tensor.transpose(psT1[:], h1c[:], eyesb[:])
            h1r = ap.tile([2, 128], F32)
            nc.vector.tensor_copy(h1r[:], psT1[:])
            h1d = dp.tile([C1], F32)
            nc.sync.dma_start(h1d[:].rearrange("(a b) -> a b", b=128), h1r[:])
            h1g = dp.tile([H1], F32, addr_space="Shared")
            nc.gpsimd.collective_compute(
                "AllGather", mybir.AluOpType.bypass,
                ins=[h1d[:].opt()], outs=[h1g[:].opt()],
                replica_groups=RG,
            )
            h1rows = ap.tile([16, 128], F32)
            nc.sync.dma_start(h1rows[:], h1g[:].rearrange("(a b) -> a b", b=128))
            psT2 = psA.tile([128, 16], F32, tag="psA", name="psT2")
            nc.tensor.transpose(psT2[:], h1rows[:], eyesb[0:16, 0:16])
            h1cols = ap.tile([128, 16], BF16)
            nc.vector.tensor_copy(h1cols[:], psT2[:])

            # ---- L2: h2 = tanh(h1 @ W2 + b2), full width, replicated
            ps2 = psA.tile([128, 4], F32, tag="psA", name="ps2")
            for m in range(4):
                for kc in range(16):
                    nc.tensor.matmul(
                        ps2[:, m : m + 1],
                        w2sb[:, (m * 16 + kc) * 128 : (m * 16 + kc + 1) * 128],
                        h1cols[:, kc : kc + 1],
                        start=(kc == 0),
                        stop=(kc == 15),
                    )
            h2cols = ap.tile([128, 4], BF16)
            for m in range(4):
                nc.scalar.activation(
                    h2cols[:, m : m + 1], ps2[:, m : m + 1], Tanh,
                    bias=b2sb[:, m : m + 1], scale=1.0 / FP8_SCALE,
                )

            # ---- L3: z = h2 @ W3 + b3 (no tanh) ----
            pz = psA.tile([128, 1], F32, tag="psA", name="pz")
            for kc in range(4):
                nc.tensor.matmul(
                    pz[:], w3sb[:, 128 * kc : 128 * (kc + 1)], h2cols[:, kc : kc + 1],
                    start=(kc == 0), stop=(kc == 3),
                )
            zcol = ap.tile([128, 1], BF16)
            nc.scalar.activation(
                zcol[:], pz[:], mybir.ActivationFunctionType.Identity,
                bias=b3sb[:], scale=1.0 / FP8_SCALE,
            )

            # ---- L4: h4 = tanh(z @ W4 + b4) ----
            ps4 = psA.tile([128, 4], F32, tag="psA", name="ps4")
            for m in range(4):
                nc.tensor.matmul(
                    ps4[:, m : m + 1], w4sb[:, 128 * m : 128 * (m + 1)], zcol[:],
                    start=True, stop=True,
                )
            h4cols = ap.tile([128, 4], BF16)
            for m in range(4):
                nc.scalar.activation(
                    h4cols[:, m : m + 1], ps4[:, m : m + 1], Tanh,
                    bias=b4sb[:, m : m + 1], scale=1.0 / FP8_SCALE,
                )

            # ---- L5: h5 full [128, 16], replicated ----
            ps5 = ps5p.tile([128, 16], F32, tag="ps5", name="ps5")
            for m in range(16):
                for kc in range(4):
                    nc.tensor.matmul(
                        ps5[:, m : m + 1],
                        w5sb[:, (m * 4 + kc) * 128 : (m * 4 + kc + 1) * 128],
                        h4cols[:, kc : kc + 1],
                        start=(kc == 0),
                        stop=(kc == 3),
                    )
            h5cols = ap.tile([128, 16], BF16)
            for m in range(16):
                nc.scalar.activation(
                    h5cols[:, m : m + 1], ps5[:, m : m + 1], Tanh,
                    bias=b5sb[:, m : m + 1], scale=1.0 / FP8_SCALE,
                )

            # ---- L6: y on own haloed span; q-major chunks, contiguous chains
            ps6 = ps6p.tile([128, NQ], F32, tag="ps6", name="ps6")
            for g in range(6):
                for ql in range(3):
                    q = 3 * g + ql
                    for kc in range(KC6):
                        nc.tensor.matmul(
                            ps6[:, q : q + 1],
                            w6sb[g][:, (ql * KC6 + kc) * 128 : (ql * KC6 + kc + 1) * 128],
                            h5cols[:, kc : kc + 1],
                            start=(kc == 0),
                            stop=(kc == KC6 - 1),
                        )
            yv = ap.tile([128, NQ], F32)
            nc.vector.tensor_tensor(yv[:], ps6[:], b6sb[:], ADD)

            # ---- span -> [18, 128] chunk rows, then the windowed [18, 256]
            ptY = psA.tile([NQ, 128], F32, tag="psA", name="ptY")
            nc.tensor.transpose(ptY[:], yv[:], eyesb[:])
            red = ap.tile([NQ, 128], F32)
            nc.vector.tensor_copy(red[:], ptY[:])
            hf = ap.tile([NQ, 256], F32)
            nc.vector.memset(hf[0:1, 0:128], 0.0)
            nc.vector.tensor_copy(hf[:, 128:256], red[:])
            nc.sync.dma_start(hf[1:NQ, 0:128], red[0 : NQ - 1, :])

            hfe = ap.tile([NQ, 256], F32)
            nc.scalar.activation(hfe[:], hf[:], Exp, scale=1.0 / FP8_SCALE)
            sf = ap.tile([NQ, 256], F32)
            nc.vector.tensor_tensor_scan(sf[:], mf[:], hfe[:], 0.0, MUL, ADD)

            e_ap = hfe[:, 128:256]
            pt1 = psA.tile([128, NQ], F32, tag="psA", name="pt1")
            nc.tensor.transpose(pt1[:], e_ap, j18sb[:])
            ct1 = ap.tile([128, NQ], F32)
            nc.vector.tensor_copy(ct1[:], pt1[:])
            pt2 = psA.tile([NQ, 128], F32, tag="psA", name="pt2")
            nc.tensor.transpose(pt2[:], ct1[:], jsb[:])
            er = ap.tile([NQ, 128], F32)
            nc.vector.tensor_copy(er[:], pt2[:])
            psh = psA.tile([NQ, 128], F32, tag="psA", name="psh")
            nc.tensor.matmul(psh[:], sh18sb[:], er[:], start=True, stop=True)
            sr1 = ap.tile([NQ, 128], F32)
            nc.vector.tensor_tensor_scan(sr1[:], mr[:, 0:128], psh[:], 0.0, MUL, ADD)
            sr = ap.tile([NQ, 128], F32)
            nc.vector.tensor_tensor_scan(
                sr[:], mr[:, 128:256], er[:], sr1[:, 127:128], MUL, ADD
            )
            pt3 = psA.tile([128, NQ], F32, tag="psA", name="pt3")
            nc.tensor.transpose(pt3[:], sr[:], j18sb[:])
            ct3 = ap.tile([128, NQ], F32)
            nc.vector.tensor_copy(ct3[:], pt3[:])
            pt4 = psA.tile([NQ, 128], F32, tag="psA", name="pt4")
            nc.tensor.transpose(pt4[:], ct3[:], jsb[:])
            dd = ap.tile([NQ, 128], F32)
            nc.vector.tensor_tensor(dd[:], sf[:, 128:256], pt4[:], ADD)
            nc.vector.tensor_tensor(dd[:], dd[:], e_ap, SUB)
            rr = ap.tile([NQ, 128], F32)
            nc.vector.reciprocal(rr[:], dd[:])
            outt = ap.tile([NQ, 128], F32)
            nc.vector.tensor_tensor(outt[:], e_ap, rr[:], MUL)
            nc.gpsimd.dma_start(
                out_ext[:].rearrange("(a b) -> a b", b=128), outt[1 : NQ - 1, :]
            )

    # Strip Tile-inserted waits from the DUMMY AllGather only, so every rank
    # joins it at launch (Tile schedules it after all input DMAs otherwise,
    # making the tail drain wait for the slowest rank's late join).
    for f in nc.m.functions:
        for b in f.blocks:
            for inst in b.instructions:
                if (
                    isinstance(inst, mybir.InstCollectiveCompute)
                    and inst.name in dummy_cc_names
                ):
                    si = inst.sync_info
                    if si is not None:
                        si.on_wait = []
    _split_sync_waits(nc)
    lower_extended_insts(nc)
    return nc


def _prep_inputs(x, W1, b1, W2, b2, W3, b3, W4, b4, W5, b5, W6, b6, segment_ids):
    """Host-side sharding + layout permutation. Returns in_maps (one per core)."""
    x = np.ascontiguousarray(x, np.float32)
    seg = np.asarray(segment_ids)

    start = np.ones(DS, bool)
    start[1:] = seg[1:] != seg[:-1]
    end = np.ones(DS, bool)
    end[:-1] = seg[:-1] != seg[1:]
    seg_len = np.diff(np.concatenate([np.where(start)[0], [DS]]))
    assert seg_len.max() <= 128, f"segment too long for halo scan: {seg_len.max()}"

    eye = np.eye(128, dtype=np.float32)
    jr18 = np.eye(NQ, dtype=np.float32)[::-1].copy()
    jrev = eye[::-1].copy()
    shm18 = np.zeros((NQ, NQ), np.float32)
    shm18[np.arange(NQ - 1), np.arange(1, NQ)] = 1.0

    xTh = np.ascontiguousarray(x.reshape(128, 128).T).astype(mybir.dt.np(BF16))

    W1 = np.asarray(W1, np.float32)
    W2 = np.asarray(W2, np.float32)
    W3 = np.asarray(W3, np.float32)
    W4 = np.asarray(W4, np.float32)
    W5 = np.asarray(W5, np.float32)
    W6 = np.asarray(W6, np.float32)
    b1 = np.asarray(b1, np.float32)
    b5 = np.asarray(b5, np.float32)
    b6 = np.asarray(b6, np.float32)
    f8 = mybir.dt.np(FP8)

    # replicated weights, weight-stationary layouts (see param comments)
    w2h = np.ascontiguousarray(
        (W2 * FP8_SCALE).reshape(16, 128, 4, 128).transpose(1, 2, 0, 3).reshape(128, 8192)
    ).astype(f8)
    w3h = np.ascontiguousarray(
        (W3 * FP8_SCALE).reshape(4, 128, 128).transpose(1, 0, 2).reshape(128, 512)
    ).astype(f8)
    w4h = np.ascontiguousarray(W4 * FP8_SCALE).astype(f8)
    w5h = np.ascontiguousarray(
        (W5 * FP8_SCALE).reshape(4, 128, 16, 128).transpose(1, 2, 0, 3).reshape(128, 8192)
    ).astype(f8)
    b2cv = np.ascontiguousarray(np.asarray(b2, np.float32).reshape(4, 128).T)
    b3cv = np.ascontiguousarray(np.asarray(b3, np.float32).reshape(1, 128).T)
    b4cv = np.ascontiguousarray(np.asarray(b4, np.float32).reshape(4, 128).T)
    b5cv = np.ascontiguousarray(b5.reshape(16, 128).T)

    in_maps = []
    for c in range(NC):
        # L1 weight-stationary layout, m-major: chunk ct covers the
        # (m = ct//2, kc = 64*(ct%2) + kl) quarter; col = kl*128 + p
        w1s = W1[:, C1 * c : C1 * (c + 1)] * FP8_SCALE   # [16384, 256]
        w1h = np.ascontiguousarray(
            w1s.reshape(2, 64, 128, 2, 128)   # [cthalf, kl, k, m, p]
            .transpose(3, 0, 2, 1, 4)         # [m, cthalf, k, kl, p]
            .reshape(4, 128, 64 * 128)
        ).astype(f8)

        # L6 column-shard: own haloed span; q-major chunk layout
        cols = (np.arange(SLICE * c - HALO, SLICE * (c + 1) + HALO)) % DS
        w6span = W6[:, cols] * FP8_SCALE                  # [2048, 2304]
        w6h = np.ascontiguousarray(
            w6span.reshape(KC6, 128, 6, 3, 128)           # [kc, k, g, ql, p]
            .transpose(2, 1, 3, 0, 4)                     # [g, k, ql, kc, p]
            .reshape(6, 128, 3 * KC6 * 128)
        ).astype(f8)
        b6qv = np.ascontiguousarray(
            (b6[cols] * FP8_SCALE).reshape(NQ, 128).T     # [p, q]
        )

        # per-core segmented-softmax masks over the haloed span of slice c
        st = start[cols].reshape(NQ, 128)
        en = end[cols].reshape(NQ, 128)
        m_own = (~st).astype(np.float32)
        mfh = np.zeros((NQ, 256), np.float32)
        mfh[1:, 0:128] = m_own[0 : NQ - 1]
        mfh[:, 128:256] = m_own
        m_rot = (~en).astype(np.float32)[::-1, ::-1]
        mrh = np.zeros((NQ, 256), np.float32)
        mrh[1:, 0:128] = m_rot[0 : NQ - 1]
        mrh[:, 128:256] = m_rot

        b1s = b1[C1 * c : C1 * (c + 1)]
        in_maps.append(
            {
                "xT": xTh,
                "w1": w1h,
                "w2": w2h,
                "w3": w3h,
                "w4": w4h,
                "w5": w5h,
                "w6": w6h,
                "b1c": np.ascontiguousarray(b1s.reshape(2, 128).T),
                "b2c": b2cv,
                "b3c": b3cv,
                "b4c": b4cv,
                "b5c": b5cv,
                "b6q": b6qv,
                "eye": eye,
                "jrev": jrev,
                "jr18": jr18,
                "shm18": shm18,
                "mf": mfh,
                "mr": mrh,
            }
        )
    return in_maps


_GRAPH_CACHE = {}


def _get_graph():
    if "nc" not in _GRAPH_CACHE:
        _GRAPH_CACHE["nc"] = build_graph()
    return _GRAPH_CACHE["nc"]


def kernel(**inputs) -> np.ndarray:
    in_maps = _prep_inputs(**inputs)
    nc = _get_graph()
    res = run_bass_kernel_spmd(nc, in_maps, core_ids=list(range(NC)))
    return np.concatenate(
        [np.asarray(res.results[c]["out"], np.float32) for c in range(NC)]
    )


# revision 23
# speedup vs baseline: 2.0309x; 1.3331x over previous
"""Trainium2 Bass kernel for nn_AutoEncoder (segment_reduce).

6-layer MLP autoencoder on a single 16384-vector + segmented softmax over
1024 contiguous segments, distributed over 8 NeuronCores.

v5 design (vs the v2 CC-AllReduce baseline at ~161us):
  * All mat-vec layers run WEIGHT-STATIONARY: lhsT = 128x128 fp8 weight
    block (Fast Weight Load eligible), rhs = the activation column
    [128, 1]. A weight-load-dominated fp8 matmul is ~40ns vs N cycles
    when the weights stream as the moving operand. fp8-lhsT x bf16-rhs
    is numerically EXACT (micro-verified); but each PSUM column's
    accumulation chain must be CONTIGUOUS — interleaving two open chains
    corrupts the accumulation (~6e-2 rel err, micro-verified). Weight
    layouts are m-major / q-major so chains stay chunk-streamable.
  * ONE cross-core sync point: a CC AllGather of the h1 shard (256 f32,
    1KB). The middle layers W2..W5 are replicated (fp8, +2.2MB DMA,
    hidden under the W1/W6 stream), so h2..h5 are computed fully locally
    and the decoder needs no second exchange and no XOR-delta games.
    (P2P remote_dma was tried: each broadcast frame costs ~7us in the
    SWDGE/D2D path, so two 7-frame exchanges burned ~90us.)
  * Each core computes y exactly on its own haloed 2304-column span of
    W6 (column-sharded decoder) and runs the windowed segmented softmax
    from SBUF; output is the core's own 2048-slice.
  * A dummy CC AllGather with its Tile-inserted waits stripped runs at
    launch: it forces the synchronized cross-rank dispatch and absorbs
    ncfw's first-collective wake latency in parallel with the stream.
"""

import sys

if "/opt/trn_rl_repo" not in sys.path:
    sys.path.insert(0, "/opt/trn_rl_repo")

import numpy as np

import concourse.bass as bass
import concourse.mybir as mybir
import concourse.tile as tile
from concourse import library_config
from concourse.bass_utils import run_bass_kernel_spmd
from concourse.library_overlay import lower_extended_insts
from concourse.tile_rust import add_dep_helper

DS = 16384
H1, H2, H3 = 2048, 512, 128
NC = 8
C1 = H1 // NC       # 256  h1 shard
SLICE = DS // NC    # 2048 y-slice per core
HALO = 128
SPAN = SLICE + 2 * HALO          # 2304
NQ = SPAN // 128                 # 18 chunks of 128
KC6 = H1 // 128                  # 16 k-chunks for L6
F32 = mybir.dt.float32
BF16 = mybir.dt.bfloat16
FP8 = mybir.dt.float8e4
FP8_SCALE = 64.0


def _split_sync_waits(nc):
    """The walrus build in this env only allows 1 sync wait on CTRL-class
    instructions (Drain/NoOp). Tile's tail drain carries one wait per live
    semaphore lane. Split excess waits onto preceding single-wait NOPs."""
    for f in nc.m.functions:
        for b in f.blocks:
            new_insts = []
            for inst in b.instructions:
                si = inst.sync_info
                if si is not None and si.on_wait and len(si.on_wait) > 1:
                    waits = list(si.on_wait)
                    head, tail = waits[:-1], waits[-1:]
                    for i, w in enumerate(head):
                        new_insts.append(
                            mybir.InstNoOp(
                                name=f"{inst.name}-ws{i}",
                                engine=inst.engine,
                                bass_nofuse=True,
                                sync_info=mybir.SyncInfo(on_wait=[w], on_update=[]),
                            )
                        )
                    si.on_wait = tail
                new_insts.append(inst)
            b.instructions = new_insts


def build_graph():
    nc = bass.Bass(num_swdge_queues=2)
    P = nc.declare_dram_parameter
    xT_in = P("xT", [128, 128], BF16, isOutput=False)       # xT[k, kc] = x[128kc+k]
    w1 = P("w1", [4, 128, 64 * 128], FP8, isOutput=False)   # m-major: (ct, k, kl*128+p)
    w2 = P("w2", [128, 4 * 16 * 128], FP8, isOutput=False)  # (k, (m*16+kc)*128+p)
    w3 = P("w3", [128, 4 * 128], FP8, isOutput=False)       # (k, kc*128+p)
    w4 = P("w4", [128, 4 * 128], FP8, isOutput=False)       # (k, m*128+p)
    w5 = P("w5", [128, 16 * 4 * 128], FP8, isOutput=False)  # (k, (m*4+kc)*128+p)
    w6 = P("w6", [6, 128, 3 * KC6 * 128], FP8, isOutput=False)  # (g, k, (ql*16+kc)*128+p)
    b1c = P("b1c", [128, 2], F32, isOutput=False)
    b2c = P("b2c", [128, 4], F32, isOutput=False)
    b3c = P("b3c", [128, 1], F32, isOutput=False)
    b4c = P("b4c", [128, 4], F32, isOutput=False)
    b5c = P("b5c", [128, 16], F32, isOutput=False)
    b6q = P("b6q", [128, NQ], F32, isOutput=False)          # b6 span cols * scale
    eye = P("eye", [128, 128], F32, isOutput=False)
    jrev = P("jrev", [128, 128], F32, isOutput=False)
    jr18 = P("jr18", [NQ, NQ], F32, isOutput=False)
    shm18 = P("shm18", [NQ, NQ], F32, isOutput=False)
    mf_in = P("mf", [NQ, 256], F32, isOutput=False)
    mr_in = P("mr", [NQ, 256], F32, isOutput=False)
    out_ext = P("out", [SLICE], F32, isOutput=True)

    Tanh = mybir.ActivationFunctionType.Tanh
    Exp = mybir.ActivationFunctionType.Exp
    ADD = mybir.AluOpType.add
    SUB = mybir.AluOpType.subtract
    MUL = mybir.AluOpType.mult
    RG = [list(range(NC))]
    dummy_cc_names = []
    rsem1 = nc.alloc_semaphore("rsem1")
    rsemw = nc.alloc_semaphore("rsemw")
    lsem = nc.alloc_semaphore("lsem_rdma")
    markers = {}

    with tile.TileContext(nc) as tc:
        with (
            tc.tile_pool(name="const", bufs=1) as cp,
            tc.tile_pool(name="w1p", bufs=4) as w1p,
            tc.tile_pool(name="w6p", bufs=6) as w6p,
            tc.tile_pool(name="act", bufs=1) as ap,
            tc.tile_pool(name="psA", bufs=2, space="PSUM") as psA,
            tc.tile_pool(name="ps1p", bufs=1, space="PSUM") as ps1p,
            tc.tile_pool(name="ps5p", bufs=1, space="PSUM") as ps5p,
            tc.tile_pool(name="ps6p", bufs=1, space="PSUM") as ps6p,
            tc.tile_pool(name="dram", bufs=1, space="DRAM") as dp,
        ):
            # gpsimd: load the remote_dma ucode library up front
            nc.gpsimd.load_library(library_config.remote_dma)

            # Dummy CC AllGather (waits stripped post-Tile): forces the
            # synchronized cross-rank launch and absorbs ncfw's
            # first-collective wake latency while the stream runs.
            dumin = dp.tile([8], F32)
            dumout = dp.tile([8 * NC], F32, addr_space="Shared")
            cc0 = nc.gpsimd.collective_compute(
                "AllGather", mybir.AluOpType.bypass,
                ins=[dumin[:].opt()], outs=[dumout[:].opt()],
                replica_groups=RG,
            )
            dummy_cc_names.append(cc0.ins.name)

            # Warm the SWDGE q1 remote-dma ring: frames fire on a ~7.2us
            # ring cadence with ~2 idle ticks of startup lag; three junk
            # self-sem-update frames + an early trigger eat that lag so the
            # real h1 frame fires on the first tick after its trigger.
            warm_rd = [(0, 0)] + [None] * 7
            for i in range(3):
                nc.gpsimd.remote_sem_update_broadcast(
                    rsemw, lsem, rdests=warm_rd, queue_num=1
                )
            tw = nc.gpsimd.trigger_dma(count=None, queue_num=1)

            # ---- sync HWDGE ring: x first, then the replicated encoder tail
            xT = cp.tile([128, 128], BF16)
            nc.sync.dma_start(xT[:], xT_in[:])
            eyesb = cp.tile([128, 128], F32)
            nc.sync.dma_start(eyesb[:], eye[:])
            b1sb = cp.tile([128, 2], F32)
            nc.sync.dma_start(b1sb[:], b1c[:])
            w2sb = cp.tile([128, 8192], FP8)
            nc.sync.dma_start(w2sb[:], w2[:])
            w3sb = cp.tile([128, 512], FP8)
            nc.sync.dma_start(w3sb[:], w3[:])
            b2sb = cp.tile([128, 4], F32)
            nc.sync.dma_start(b2sb[:], b2c[:])
            b3sb = cp.tile([128, 1], F32)
            nc.sync.dma_start(b3sb[:], b3c[:])
            # ---- scalar ring: decoder weights + softmax constants
            w5sb = cp.tile([128, 8192], FP8)
            nc.scalar.dma_start(w5sb[:], w5[:])
            w4sb = cp.tile([128, 512], FP8)
            nc.scalar.dma_start(w4sb[:], w4[:])
            jsb = cp.tile([128, 128], F32)
            nc.scalar.dma_start(jsb[:], jrev[:])
            j18sb = cp.tile([NQ, NQ], F32)
            nc.scalar.dma_start(j18sb[:], jr18[:])
            sh18sb = cp.tile([NQ, NQ], F32)
            nc.scalar.dma_start(sh18sb[:], shm18[:])
            mf = cp.tile([NQ, 256], F32)
            nc.scalar.dma_start(mf[:], mf_in[:])
            mr = cp.tile([NQ, 256], F32)
            nc.scalar.dma_start(mr[:], mr_in[:])
            b4sb = cp.tile([128, 4], F32)
            nc.scalar.dma_start(b4sb[:], b4c[:])
            b5sb = cp.tile([128, 16], F32)
            nc.scalar.dma_start(b5sb[:], b5c[:])
            b6sb = cp.tile([128, NQ], F32)
            nc.scalar.dma_start(b6sb[:], b6q[:])

            # ---- bulk W1 + W6 stream on SWDGE queue 0
            w1sb_l = []
            for ct in range(4):
                t = w1p.tile([128, 64 * 128], FP8, tag="w1sb", name=f"w1sb{ct}")
                nc.gpsimd.dma_start(t[:], w1[ct])
                w1sb_l.append(t)
            w6sb = []
            for g in range(6):
                t = w6p.tile([128, 3 * KC6 * 128], FP8, tag="w6", name=f"w6sb{g}")
                nc.gpsimd.dma_start(t[:], w6[g])
                w6sb.append(t)

            # ---- L1: h1 shard [128, 2] cols; m-major contiguous chains
            ps1 = ps1p.tile([128, 2], F32, tag="ps1", name="ps1")
            for ct in range(4):
                m = ct // 2
                for kl in range(64):
                    kc = 64 * (ct % 2) + kl
                    nc.tensor.matmul(
                        ps1[:, m : m + 1],
                        w1sb_l[ct][:, kl * 128 : (kl + 1) * 128],
                        xT[:, kc : kc + 1],
                        start=(ct % 2 == 0 and kl == 0),
                        stop=(ct % 2 == 1 and kl == 63),
                    )
            h1pre = ap.tile([128, 2], F32)
            nc.vector.tensor_tensor(h1pre[:], ps1[:], b1sb[:], ADD)
            h1c = ap.tile([128, 2], F32)
            nc.scalar.activation(h1c[:], h1pre[:], Tanh, scale=1.0 / FP8_SCALE)

            # ---- h1 all-gather: ONE 8-dest broadcast frame. Every core
            # (incl. self via delta 0) receives every sender's [128, 2]
            # shard; the pid-register dest offset makes each sender write
            # its OWN slot on every receiver, so slots are rank-ordered
            # regardless of the XOR-delta delivery permutation. rcv1 is
            # NEVER written locally (remote data may arrive early).
            rcv1 = cp.tile([128, 16], F32)
            pid = nc.gpsimd.partition_id()
            dst_ap = bass.AP(rcv1[:].tensor, pid * 2, [[16, 128], [1, 2]])
            br = nc.gpsimd.remote_dma_broadcast(
                dst_ap, h1c[:], rsem1, lsem,
                rdests=[(0, d) for d in range(NC)], queue_num=1,
            )
            add_dep_helper(br.ins, tw.ins, sync=False, reason="after warm trigger")
            for i in range(2):  # pad to 3 frames per trigger (ucode quirk)
                pr = nc.gpsimd.remote_sem_update_broadcast(
                    rsemw, lsem, rdests=warm_rd, queue_num=1
                )
                add_dep_helper(pr.ins, br.ins, sync=False, reason="pad after real")
            t1 = nc.gpsimd.trigger_dma(count=None, queue_num=1)
            mk1 = nc.vector.nop(nofuse=True, hint="rsem1_w")
            add_dep_helper(mk1.ins, t1.ins, sync=False, reason="wait after trigger")
            markers["h1"] = (mk1.ins.name, rsem1, 16)
            h1cols = ap.tile([128, 16], BF16)
            c1 = nc.vector.tensor_copy(h1cols[:], rcv1[:])
            add_dep_helper(c1.ins, mk1.ins, sync=False, reason="cast after sem wait")

            # ---- L2: h2 = tanh(h1 @ W2 + b2), full width, replicated
            ps2 = psA.tile([128, 4], F32, tag="psA", name="ps2")
            for m in range(4):
                for kc in range(16):
                    nc.tensor.matmul(
                        ps2[:, m : m + 1],
                        w2sb[:, (m * 16 + kc) * 128 : (m * 16 + kc + 1) * 128],
                        h1cols[:, kc : kc + 1],
                        start=(kc == 0),
                        stop=(kc == 15),
                    )
            h2pre = ap.tile([128, 4], F32)
            nc.vector.tensor_tensor(h2pre[:], ps2[:], b2sb[:], ADD)
            h2cols = ap.tile([128, 4], BF16)
            nc.scalar.activation(h2cols[:], h2pre[:], Tanh, scale=1.0 / FP8_SCALE)

            # ---- L3: z = h2 @ W3 + b3 (no tanh) ----
            pz = psA.tile([128, 1], F32, tag="psA", name="pz")
            for kc in range(4):
                nc.tensor.matmul(
                    pz[:], w3sb[:, 128 * kc : 128 * (kc + 1)], h2cols[:, kc : kc + 1],
                    start=(kc == 0), stop=(kc == 3),
                )
            zpre = ap.tile([128, 1], F32)
            nc.vector.tensor_tensor(zpre[:], pz[:], b3sb[:], ADD)
            zcol = ap.tile([128, 1], BF16)
            nc.scalar.activation(
                zcol[:], zpre[:], mybir.ActivationFunctionType.Identity,
                scale=1.0 / FP8_SCALE,
            )

            # ---- L4: h4 = tanh(z @ W4 + b4) ----
            ps4 = psA.tile([128, 4], F32, tag="psA", name="ps4")
            for m in range(4):
                nc.tensor.matmul(
                    ps4[:, m : m + 1], w4sb[:, 128 * m : 128 * (m + 1)], zcol[:],
                    start=True, stop=True,
                )
            h4pre = ap.tile([128, 4], F32)
            nc.vector.tensor_tensor(h4pre[:], ps4[:], b4sb[:], ADD)
            h4cols = ap.tile([128, 4], BF16)
            nc.scalar.activation(h4cols[:], h4pre[:], Tanh, scale=1.0 / FP8_SCALE)

            # ---- L5: h5 full [128, 16], replicated ----
            ps5 = ps5p.tile([128, 16], F32, tag="ps5", name="ps5")
            for m in range(16):
                for kc in range(4):
                    nc.tensor.matmul(
                        ps5[:, m : m + 1],
                        w5sb[:, (m * 4 + kc) * 128 : (m * 4 + kc + 1) * 128],
                        h4cols[:, kc : kc + 1],
                        start=(kc == 0),
                        stop=(kc == 3),
                    )
            h5pre = ap.tile([128, 16], F32)
            nc.vector.tensor_tensor(h5pre[:], ps5[:], b5sb[:], ADD)
            h5cols = ap.tile([128, 16], BF16)
            nc.scalar.activation(h5cols[:], h5pre[:], Tanh, scale=1.0 / FP8_SCALE)

            # ---- L6: y on own haloed span; q-major chunks, contiguous chains
            ps6 = ps6p.tile([128, NQ], F32, tag="ps6", name="ps6")
            for g in range(6):
                for ql in range(3):
                    q = 3 * g + ql
                    for kc in range(KC6):
                        nc.tensor.matmul(
                            ps6[:, q : q + 1],
                            w6sb[g][:, (ql * KC6 + kc) * 128 : (ql * KC6 + kc + 1) * 128],
                            h5cols[:, kc : kc + 1],
                            start=(kc == 0),
                            stop=(kc == KC6 - 1),
                        )
            yv = ap.tile([128, NQ], F32)
            nc.vector.tensor_tensor(yv[:], ps6[:], b6sb[:], ADD)

            # ---- span -> [18, 128] chunk rows, then the windowed [18, 256]
            ptY = psA.tile([NQ, 128], F32, tag="psA", name="ptY")
            nc.tensor.transpose(ptY[:], yv[:], eyesb[:])
            red = ap.tile([NQ, 128], F32)
            nc.vector.tensor_copy(red[:], ptY[:])
            hf = ap.tile([NQ, 256], F32)
            nc.vector.memset(hf[0:1, 0:128], 0.0)
            nc.vector.tensor_copy(hf[:, 128:256], red[:])
            nc.sync.dma_start(hf[1:NQ, 0:128], red[0 : NQ - 1, :])

            hfe = ap.tile([NQ, 256], F32)
            nc.scalar.activation(hfe[:], hf[:], Exp, scale=1.0 / FP8_SCALE)
            sf = ap.tile([NQ, 256], F32)
            nc.vector.tensor_tensor_scan(sf[:], mf[:], hfe[:], 0.0, MUL, ADD)

            e_ap = hfe[:, 128:256]
            pt1 = psA.tile([128, NQ], F32, tag="psA", name="pt1")
            nc.tensor.transpose(pt1[:], e_ap, j18sb[:])
            ct1 = ap.tile([128, NQ], F32)
            nc.vector.tensor_copy(ct1[:], pt1[:])
            pt2 = psA.tile([NQ, 128], F32, tag="psA", name="pt2")
            nc.tensor.transpose(pt2[:], ct1[:], jsb[:])
            er = ap.tile([NQ, 128], F32)
            nc.vector.tensor_copy(er[:], pt2[:])
            psh = psA.tile([NQ, 128], F32, tag="psA", name="psh")
            nc.tensor.matmul(psh[:], sh18sb[:], er[:], start=True, stop=True)
            sr1 = ap.tile([NQ, 128], F32)
            nc.vector.tensor_tensor_scan(sr1[:], mr[:, 0:128], psh[:], 0.0, MUL, ADD)
            sr = ap.tile([NQ, 128], F32)
            nc.vector.tensor_tensor_scan(
                sr[:], mr[:, 128:256], er[:], sr1[:, 127:128], MUL, ADD
            )
            pt3 = psA.tile([128, NQ], F32, tag="psA", name="pt3")
            nc.tensor.transpose(pt3[:], sr[:], j18sb[:])
            ct3 = ap.tile([128, NQ], F32)
            nc.vector.tensor_copy(ct3[:], pt3[:])
            pt4 = psA.tile([NQ, 128], F32, tag="psA", name="pt4")
            nc.tensor.transpose(pt4[:], ct3[:], jsb[:])
            dd = ap.tile([NQ, 128], F32)
            nc.vector.tensor_tensor(dd[:], sf[:, 128:256], pt4[:], ADD)
            nc.vector.tensor_tensor(dd[:], dd[:], e_ap, SUB)
            rr = ap.tile([NQ, 128], F32)
            nc.vector.reciprocal(rr[:], dd[:])
            outt = ap.tile([NQ, 128], F32)
            nc.vector.tensor_tensor(outt[:], e_ap, rr[:], MUL)
            nc.gpsimd.dma_start(
                out_ext[:].rearrange("(a b) -> a b", b=128), outt[1 : NQ - 1, :]
            )

    # Strip Tile-inserted waits from the DUMMY AllGather only, so every rank
    # joins it at launch (Tile schedules it after all input DMAs otherwise,
    # making the tail drain wait for the slowest rank's late join).
    for f in nc.m.functions:
        for b in f.blocks:
            for inst in b.instructions:
                if (
                    isinstance(inst, mybir.InstCollectiveCompute)
                    and inst.name in dummy_cc_names
                ):
                    si = inst.sync_info
                    if si is not None:
                        si.on_wait = []
    # inject remote-sem waits on the marker nops (invisible to Tile's sim)
    want = {v[0]: (v[1], v[2]) for v in markers.values()}
    found = 0
    for f in nc.m.functions:
        for b in f.blocks:
            for inst in b.instructions:
                if inst.name in want:
                    sem, val = want[inst.name]
                    bass.BassInstruction(inst)._wait_ge(sem, val)
                    found += 1
    assert found == len(want), f"injected {found} of {len(want)} sem waits"
    _split_sync_waits(nc)
    lower_extended_insts(nc)
    return nc


def _prep_inputs(x, W1, b1, W2, b2, W3, b3, W4, b4, W5, b5, W6, b6, segment_ids):
    """Host-side sharding + layout permutation. Returns in_maps (one per core)."""
    x = np.ascontiguousarray(x, np.float32)
    seg = np.asarray(segment_ids)

    start = np.ones(DS, bool)
    start[1:] = seg[1:] != seg[:-1]
    end = np.ones(DS, bool)
    end[:-1] = seg[:-1] != seg[1:]
    seg_len = np.diff(np.concatenate([np.where(start)[0], [DS]]))
    assert seg_len.max() <= 128, f"segment too long for halo scan: {seg_len.max()}"

    eye = np.eye(128, dtype=np.float32)
    jr18 = np.eye(NQ, dtype=np.float32)[::-1].copy()
    jrev = eye[::-1].copy()
    shm18 = np.zeros((NQ, NQ), np.float32)
    shm18[np.arange(NQ - 1), np.arange(1, NQ)] = 1.0

    xTh = np.ascontiguousarray(x.reshape(128, 128).T).astype(mybir.dt.np(BF16))

    W1 = np.asarray(W1, np.float32)
    W2 = np.asarray(W2, np.float32)
    W3 = np.asarray(W3, np.float32)
    W4 = np.asarray(W4, np.float32)
    W5 = np.asarray(W5, np.float32)
    W6 = np.asarray(W6, np.float32)
    b1 = np.asarray(b1, np.float32)
    b5 = np.asarray(b5, np.float32)
    b6 = np.asarray(b6, np.float32)
    f8 = mybir.dt.np(FP8)

    # replicated weights, weight-stationary layouts (see param comments)
    w2h = np.ascontiguousarray(
        (W2 * FP8_SCALE).reshape(16, 128, 4, 128).transpose(1, 2, 0, 3).reshape(128, 8192)
    ).astype(f8)
    w3h = np.ascontiguousarray(
        (W3 * FP8_SCALE).reshape(4, 128, 128).transpose(1, 0, 2).reshape(128, 512)
    ).astype(f8)
    w4h = np.ascontiguousarray(W4 * FP8_SCALE).astype(f8)
    w5h = np.ascontiguousarray(
        (W5 * FP8_SCALE).reshape(4, 128, 16, 128).transpose(1, 2, 0, 3).reshape(128, 8192)
    ).astype(f8)
    # biases are added in the x64-scaled PSUM domain (vector add pre-act)
    b2cv = np.ascontiguousarray(np.asarray(b2, np.float32).reshape(4, 128).T * FP8_SCALE)
    b3cv = np.ascontiguousarray(np.asarray(b3, np.float32).reshape(1, 128).T * FP8_SCALE)
    b4cv = np.ascontiguousarray(np.asarray(b4, np.float32).reshape(4, 128).T * FP8_SCALE)
    b5cv = np.ascontiguousarray(b5.reshape(16, 128).T * FP8_SCALE)

    in_maps = []
    for c in range(NC):
        # L1 weight-stationary layout, m-major: chunk ct covers the
        # (m = ct//2, kc = 64*(ct%2) + kl) quarter; col = kl*128 + p
        w1s = W1[:, C1 * c : C1 * (c + 1)] * FP8_SCALE   # [16384, 256]
        w1h = np.ascontiguousarray(
            w1s.reshape(2, 64, 128, 2, 128)   # [cthalf, kl, k, m, p]
            .transpose(3, 0, 2, 1, 4)         # [m, cthalf, k, kl, p]
            .reshape(4, 128, 64 * 128)
        ).astype(f8)

        # L6 column-shard: own haloed span; q-major chunk layout
        cols = (np.arange(SLICE * c - HALO, SLICE * (c + 1) + HALO)) % DS
        w6span = W6[:, cols] * FP8_SCALE                  # [2048, 2304]
        w6h = np.ascontiguousarray(
            w6span.reshape(KC6, 128, 6, 3, 128)           # [kc, k, g, ql, p]
            .transpose(2, 1, 3, 0, 4)                     # [g, k, ql, kc, p]
            .reshape(6, 128, 3 * KC6 * 128)
        ).astype(f8)
        b6qv = np.ascontiguousarray(
            (b6[cols] * FP8_SCALE).reshape(NQ, 128).T     # [p, q]
        )

        # per-core segmented-softmax masks over the haloed span of slice c
        st = start[cols].reshape(NQ, 128)
        en = end[cols].reshape(NQ, 128)
        m_own = (~st).astype(np.float32)
        mfh = np.zeros((NQ, 256), np.float32)
        mfh[1:, 0:128] = m_own[0 : NQ - 1]
        mfh[:, 128:256] = m_own
        m_rot = (~en).astype(np.float32)[::-1, ::-1]
        mrh = np.zeros((NQ, 256), np.float32)
        mrh[1:, 0:128] = m_rot[0 : NQ - 1]
        mrh[:, 128:256] = m_rot

        b1s = b1[C1 * c : C1 * (c + 1)]
        in_maps.append(
            {
                "xT": xTh,
                "w1": w1h,
                "w2": w2h,
                "w3": w3h,
                "w4": w4h,
                "w5": w5h,
                "w6": w6h,
                "b1c": np.ascontiguousarray(b1s.reshape(2, 128).T * FP8_SCALE),
                "b2c": b2cv,
                "b3c": b3cv,
                "b4c": b4cv,
                "b5c": b5cv,
                "b6q": b6qv,
                "eye": eye,
                "jrev": jrev,
                "jr18": jr18,
                "shm18": shm18,
                "mf": mfh,
                "mr": mrh,
            }
        )
    return in_maps


_GRAPH_CACHE = {}


def _get_graph():
    if "nc" not in _GRAPH_CACHE:
        _GRAPH_CACHE["nc"] = build_graph()
    return _GRAPH_CACHE["nc"]


def kernel(**inputs) -> np.ndarray:
    in_maps = _prep_inputs(**inputs)
    nc = _get_graph()
    res = run_bass_kernel_spmd(nc, in_maps, core_ids=list(range(NC)))
    return np.concatenate(
        [np.asarray(res.results[c]["out"], np.float32) for c in range(NC)]
    )
